# revision 9
# baseline (speedup 1.0000x reference)
"""nn_MemoryAttention TP8 Trainium2 kernel, v3.

8 NeuronCores, T-layout activations [feature, token] (512 token cols =
4 batch x 128). Attention output stays head-sharded (2 heads = 256 features
per core); wm and wo are ROW-parallel on that shard; per-chunk recurrence
needs two AllReduces (om1 = a_sh @ wm, om2 = om1 + FFN) plus a per-chunk
ReduceScatter for yo = a_sh @ wo.

v3 changes vs v2:
- All weights and activations fp16 (incl. wm/wo/aout); f32 only in psum,
  softmax/norm scalar chains.
- x/mem projection weights SBUF-resident (no per-chunk reloads).
- FFN up-projection k-tiles 0-7 computed during the AR1 wire time of
  k-tiles 8-15 ("spill" partial psums to SBUF fp16, resume + add after the
  second AR half lands). Same trick hides AR2 behind KM/VM prefills.
- v projections computed directly in token-partition layout (weight as the
  moving operand), killing the per-chunk PE transposes.
- softmax/rms epilogues: psum-accumulated denominators, broadcast first,
  then reciprocal_approx_fast on the [128, 512] tile (the serial [1,512]
  DVE reciprocal was 3.3us each).
- yo ReduceScatter queued between AR1 and AR2 so it never delays the
  om-recurrence collectives.
"""
import numpy as np

DIM = 2048; NH = 16; HD = 128; MEM = 128; SEQ = 2048; B = 4; HID = 5632
EPS = 1e-5
NC = 8
HPC = NH // NC          # 2 heads per core
FPC = DIM // NC         # 256 features per core
HIDP = 768              # padded per-core FFN hidden (704 -> 768)
NCH = SEQ // MEM        # 16 chunks
TOK = B * MEM           # 512 token columns
KT = DIM // 128         # 16 feature k-tiles
HKT = HIDP // 128       # 6 hidden k-tiles
MASKV = -60.0

_RUNTIME = {}
COLL = True   # False: replace collectives with local DMAs (timing diagnostic)


def _head_perm():
    p = np.concatenate([np.arange(0, HD, 2), np.arange(1, HD, 2)])
    return np.concatenate([h * HD + p for h in range(NH)])


def _slab(w):
    """Column-shard weight [2048, C] -> [C//128, 128, KT, 128] (stationary)."""
    C = w.shape[1]
    return np.ascontiguousarray(
        w.reshape(KT, 128, C // 128, 128).transpose(2, 1, 0, 3))


def _slab_row(w):
    """Row-shard weight [256, 2048] -> [2, 128, 16, 128] (k-major)."""
    return np.ascontiguousarray(w.reshape(2, 128, 16, 128))


def _slab_mov(w):
    """Moving-operand weight [2048, 256] -> [KT, 128, 256]."""
    return np.ascontiguousarray(w.reshape(KT, 128, 256))


def _slab_w2(w2):
    """[768, 2048] -> [16, 128, HKT, 128]."""
    return np.ascontiguousarray(
        w2.reshape(HKT, 128, 16, 128).transpose(2, 1, 0, 3))


def _trunc22(x):
    u = np.ascontiguousarray(x, np.float32).view(np.uint32)
    return ((u + np.uint32(0x200)) & np.uint32(0xFFFFFC00)).view(np.float32)


def _prepare(inputs):
    perm = _head_perm()
    scale = 1.0 / np.sqrt(HD)
    fwv = np.asarray(inputs["ffn_norm_w"], np.float32)
    mwv = np.asarray(inputs["mem_norm_w"], np.float32)
    wq = np.asarray(inputs["wq"])[:, perm] * scale
    wk = np.asarray(inputs["wk"])[:, perm]
    wkm = (mwv[:, None] * np.asarray(inputs["wkm"]))[:, perm]
    wvm = mwv[:, None] * np.asarray(inputs["wvm"])
    wv = np.asarray(inputs["wv"])
    wm = np.asarray(inputs["wm"]); wo = np.asarray(inputs["wo"])
    w1 = np.zeros((DIM, NC * HIDP), np.float32)
    w3 = np.zeros((DIM, NC * HIDP), np.float32)
    w2 = np.zeros((NC * HIDP, DIM), np.float32)
    w1s = fwv[:, None] * np.asarray(inputs["w1"])
    w3s = fwv[:, None] * np.asarray(inputs["w3"])
    for c in range(NC):
        w1[:, c * HIDP:c * HIDP + 704] = w1s[:, c * 704:(c + 1) * 704]
        w3[:, c * HIDP:c * HIDP + 704] = w3s[:, c * 704:(c + 1) * 704]
        w2[c * HIDP:c * HIDP + 704] = np.asarray(inputs["w2"])[c * 704:(c + 1) * 704]
    fc = np.asarray(inputs["freqs_cos"]); fs = np.asarray(inputs["freqs_sin"])
    cos_mem = _trunc22(np.tile(fc[0:MEM].T, (1, B)))      # [64, 512]
    sin_mem = _trunc22(np.tile(fs[0:MEM].T, (1, B)))
    cos_x = _trunc22(np.tile(fc[MEM:2 * MEM].T, (1, B)))
    sin_x = _trunc22(np.tile(fs[MEM:2 * MEM].T, (1, B)))
    # causal mask for chunk-key columns only, transposed: [128 k, 4 b, 128 q]
    mask1 = np.zeros((MEM, MEM), np.float32)
    for i in range(MEM):
        mask1[i, i + 1:] = MASKV
    maskc = np.ascontiguousarray(
        np.broadcast_to(mask1.T[:, None, :], (MEM, B, MEM))).astype(np.float16)
    x = np.asarray(inputs["x"])
    xT = np.ascontiguousarray(
        x.reshape(B, NCH, MEM, DIM).transpose(1, 3, 0, 2)
        .reshape(NCH, DIM, TOK)).astype(np.float16)
    om0 = np.asarray(inputs["origin_mem"])
    omT0 = om0.transpose(2, 0, 1).reshape(DIM, TOK)
    in_maps = []
    for c in range(NC):
        hsl = slice(c * FPC, (c + 1) * FPC)
        hidsl = slice(c * HIDP, (c + 1) * HIDP)
        in_maps.append({
            "WM": _slab_row(wm[hsl, :]).astype(np.float16),
            "WO": _slab_row(wo[hsl, :]).astype(np.float16),
            "WKM": _slab(wkm[:, hsl]).astype(np.float16),
            "WVM": _slab_mov(wvm[:, hsl]).astype(np.float16),
            "WQ": _slab(wq[:, hsl]).astype(np.float16),
            "WK": _slab(wk[:, hsl]).astype(np.float16),
            "WV": _slab_mov(wv[:, hsl]).astype(np.float16),
            "W1": _slab(w1[:, hidsl]).astype(np.float16),
            "W3": _slab(w3[:, hidsl]).astype(np.float16),
            "W2": _slab_w2(w2[hidsl, :]).astype(np.float16),
            "XT": xT,
            "OM0T": omT0[c * FPC:(c + 1) * FPC].reshape(2, 128, TOK)
                    .astype(np.float16),
            "COSM": cos_mem.astype(np.float16), "SINM": sin_mem.astype(np.float16),
            "COSX": cos_x.astype(np.float16), "SINX": sin_x.astype(np.float16),
            "MASKC": maskc,
        })
    return in_maps


def _build():
    import concourse.bacc as bacc
    import concourse.tile as tile
    import concourse.mybir as mybir
    from concourse.masks import make_identity
    from contextlib import ExitStack

    dt = mybir.dt
    AluOp = mybir.AluOpType
    AFT = mybir.ActivationFunctionType
    f32, f32r, f16 = dt.float32, dt.float32r, dt.float16

    nc = bacc.Bacc("TRN2", target_bir_lowering=False, debug=False,
                   num_devices=NC)

    def din(name, shape, dtype=f16):
        return nc.dram_tensor(name, shape, dtype, kind="ExternalInput")

    WM = din("WM", [2, 128, KT, 128]); WO = din("WO", [2, 128, KT, 128])
    WKM = din("WKM", [2, 128, KT, 128])
    WVM = din("WVM", [KT, 128, 256])
    WQ = din("WQ", [2, 128, KT, 128]); WK = din("WK", [2, 128, KT, 128])
    WV = din("WV", [KT, 128, 256])
    W1 = din("W1", [HKT, 128, KT, 128])
    W3 = din("W3", [HKT, 128, KT, 128])
    W2 = din("W2", [KT, 128, HKT, 128])
    XT = din("XT", [NCH, DIM, TOK])
    OM0T = din("OM0T", [2, 128, TOK])
    COSM = din("COSM", [64, TOK]); SINM = din("SINM", [64, TOK])
    COSX = din("COSX", [64, TOK]); SINX = din("SINX", [64, TOK])
    MASKC = din("MASKC", [MEM, B, MEM])
    YO = nc.dram_tensor("YO", [NCH, FPC, TOK], f16, kind="ExternalOutput")

    rg = [list(range(NC))]

    with tile.TileContext(nc) as tc:
        es = ExitStack()
        const = es.enter_context(tc.tile_pool(name="const", bufs=1))
        wres = es.enter_context(tc.tile_pool(name="wres", bufs=1))
        w13r = es.enter_context(tc.tile_pool(name="w13r", bufs=4))
        w2r = es.enter_context(tc.tile_pool(name="w2r", bufs=2))
        ompool = es.enter_context(tc.tile_pool(name="ompool", bufs=1))
        xpool = es.enter_context(tc.tile_pool(name="xpool", bufs=1))
        gpool = es.enter_context(tc.tile_pool(name="gpool", bufs=1))
        sppool = es.enter_context(tc.tile_pool(name="sppool", bufs=1))
        qkpool = es.enter_context(tc.tile_pool(name="qkpool", bufs=1))
        vpool = es.enter_context(tc.tile_pool(name="vpool", bufs=1))
        stage = es.enter_context(tc.tile_pool(name="stage", bufs=2))
        scr = es.enter_context(tc.tile_pool(name="scr", bufs=2))
        smol = es.enter_context(tc.tile_pool(name="smol", bufs=2))
        aoutp = es.enter_context(tc.tile_pool(name="aoutp", bufs=2))
        dram = es.enter_context(tc.tile_pool(name="dram", bufs=1, space="DRAM"))
        psA = es.enter_context(tc.tile_pool(name="psA", bufs=3, space="PSUM"))
        psF = es.enter_context(tc.tile_pool(name="psF", bufs=4, space="PSUM"))
        psN = es.enter_context(tc.tile_pool(name="psN", bufs=1, space="PSUM"))

        # ---- constants
        cosm = const.tile([64, TOK], f16); nc.sync.dma_start(cosm[:], COSM[:])
        sinm = const.tile([64, TOK], f16); nc.sync.dma_start(sinm[:], SINM[:])
        cosx = const.tile([64, TOK], f16); nc.sync.dma_start(cosx[:], COSX[:])
        sinx = const.tile([64, TOK], f16); nc.sync.dma_start(sinx[:], SINX[:])
        maskc = const.tile([MEM, B, MEM], f16)
        nc.sync.dma_start(maskc[:], MASKC[:])
        scratch32 = const.tile([128, 128], f32)
        nc.vector.memset(scratch32[:], 1.0)
        ones16 = const.tile([128, 1], f16)
        nc.vector.tensor_copy(ones16[:], scratch32[:, 0:1])
        onesrow = const.tile([1, 128], f32r)
        nc.vector.tensor_copy(onesrow[:], scratch32[0:1, :])
        ident32 = const.tile([128, 128], f32)
        make_identity(nc, ident32)
        epst = const.tile([128, 1], f32)
        nc.vector.memset(epst[:], EPS)

        # ---- resident weights
        def wload(Wd, n_slabs, tag):
            w = wres.tile([128, n_slabs, KT, 128], f16, tag=tag, name=tag)
            for n in range(n_slabs):
                nc.sync.dma_start(w[:, n], Wd[n])
            return w

        wmr = wload(WM, 2, "wmr"); wor = wload(WO, 2, "wor")
        wqr = wload(WQ, 2, "wqr"); wkr = wload(WK, 2, "wkr")
        wkmr = wload(WKM, 2, "wkmr")
        wvr = wres.tile([128, KT, 256], f16, tag="wvr", name="wvr")
        nc.sync.dma_start(wvr[:], WV[:].rearrange("k p f -> p k f"))
        wvmr = wres.tile([128, KT, 256], f16, tag="wvmr", name="wvmr")
        nc.sync.dma_start(wvmr[:], WVM[:].rearrange("k p f -> p k f"))

        a_sh = aoutp.tile([128, 2, TOK], f16, tag="aout", name="a_init")
        nc.sync.dma_start(a_sh[:], OM0T[:].rearrange("k p t -> p k t"))

        def mm(p, lhsT, rhs, start, stop):
            nc.tensor.matmul(p, lhsT, rhs, start=start, stop=stop)

        def rowproj_stage(Wr, a_tile, dsts):
            """Row-parallel 256->2048 proj of a_tile [128,2,TOK] fp16; psum
            quartets staged fp16 into the DRAM dsts (list of (ap, nq))."""
            qi = 0
            for dst, nq in dsts:
                for q in range(nq):
                    stq = stage.tile([128, 4, TOK], f16, tag="stage",
                                     name="stq")
                    for j in range(4):
                        n = 4 * qi + j
                        p = psA.tile([128, TOK], f32, tag="pa", name="pn")
                        mm(p[:], Wr[:, 0, n, :], a_tile[:, 0, :], True, False)
                        mm(p[:], Wr[:, 1, n, :], a_tile[:, 1, :], False, True)
                        if j % 2 == 0:
                            nc.scalar.copy(stq[:, j, :], p[:])
                        else:
                            nc.vector.tensor_copy(stq[:, j, :], p[:])
                    nc.sync.dma_start(
                        dst[q * 4 * 128:(q + 1) * 4 * 128, :]
                        .rearrange("(j p) t -> p j t", p=128),
                        stq[:])
                    qi += 1

        def allreduce(in_d, out_d):
            if COLL:
                nc.gpsimd.collective_compute(
                    "AllReduce", AluOp.add, replica_groups=rg,
                    ins=[in_d[:].opt()], outs=[out_d[:].opt()])
            else:
                nc.sync.dma_start(out_d[:], in_d[:])

        def reduce_scatter(in_d, out_ap, tag):
            rsout = dram.tile([FPC, TOK], f16, name=f"rsout_{tag}")
            if COLL:
                nc.gpsimd.collective_compute(
                    "ReduceScatter", AluOp.add, replica_groups=rg,
                    ins=[in_d[:].opt()], outs=[rsout[:].opt()])
            else:
                nc.sync.dma_start(rsout[:], in_d[0:FPC, :])
            nc.scalar.dma_start(out_ap, rsout[:])

        def bc_from_psum(ssq_ps, kind, name):
            """[1,TOK] psum -> broadcast [128,TOK] -> rstd (Sqrt+recip) or
            plain reciprocal; returns [128,TOK] f32 SBUF tile."""
            r = smol.tile([1, TOK], f32r, tag="smr", name=f"r_{name}", bufs=2)
            nc.vector.tensor_copy(r[:], ssq_ps[:])
            pbc = psN.tile([128, TOK], f32, tag="pn", name=f"pbc_{name}")
            mm(pbc[:], onesrow[:], r[:], True, True)
            bc = scr.tile([128, TOK], f32, tag=f"bc_{kind}", name=f"bc_{name}",
                          bufs=1 if kind == "rstd" else 2)
            if kind == "rstd":
                sq = scr.tile([128, TOK], f32, tag="sqt", name=f"sq_{name}",
                              bufs=1)
                nc.scalar.activation(sq[:], pbc[:], AFT.Sqrt,
                                     bias=epst[:], scale=1.0 / DIM)
                nc.vector.reciprocal_approx_fast(bc[:], sq[:])
            else:
                nc.vector.reciprocal_approx_fast(bc[:], pbc[:])
            return bc

        def rope2(dst_fn, src01, cosT, sinT):
            """src01: two psum tiles [128, TOK]; dst_fn(h, ri) -> fp16 AP."""
            for h in range(2):
                ph = src01[h]
                r, i = ph[0:64, :], ph[64:128, :]
                t1 = scr.tile([64, TOK], f32, tag="t1", name="t1", bufs=2)
                t2 = scr.tile([64, TOK], f32, tag="t1", name="t2", bufs=2)
                nc.vector.tensor_mul(t1[:], r, cosT[:])
                nc.vector.tensor_mul(t2[:], i, sinT[:])
                nc.vector.tensor_sub(dst_fn(h, 0), t1[:], t2[:])
                t3 = scr.tile([64, TOK], f32, tag="t1", name="t3", bufs=2)
                t4 = scr.tile([64, TOK], f32, tag="t1", name="t4", bufs=2)
                nc.vector.tensor_mul(t3[:], r, sinT[:])
                nc.vector.tensor_mul(t4[:], i, cosT[:])
                nc.vector.tensor_add(dst_fn(h, 1), t3[:], t4[:])

        def colproj2_psums(Wr, rhs_tile, pool, tag, k0, k1, ps=None,
                           stop=True):
            """2048 -> 256 col proj over k in [k0,k1); returns 2 psum tiles."""
            outs = []
            for n in range(2):
                if ps is None:
                    p = pool.tile([128, TOK], f32, tag=tag, name=f"pp{n}")
                else:
                    p = ps[n]
                for k in range(k0, k1):
                    mm(p[:], Wr[:, n, k, :], rhs_tile[:, k, :], k == k0 and k0 == 0,
                       stop and k == k1 - 1)
                outs.append(p)
            return outs

        def vproj_tok(lhs_tile, Wr, v, bs, k0, k1, ps=None, stop=True):
            """v[tok, b, 256] += x[k-tile, b-block]^T @ W[k-tile] over k."""
            outs = []
            for bi, b in enumerate(bs):
                if ps is None:
                    p = psA.tile([128, 256], f32, tag="pa", name=f"pv{b}")
                else:
                    p = ps[bi]
                for k in range(k0, k1):
                    mm(p[:], lhs_tile[:, k, b * 128:(b + 1) * 128],
                       Wr[:, k, :], k == k0 and k0 == 0, stop and k == k1 - 1)
                outs.append(p)
                if stop:
                    nc.scalar.copy(v[:, b, :], p[:])
            return outs

        # DRAM comm buffers
        def mk_arbufs(tag):
            ins = [dram.tile([DIM // 2, TOK], f16, name=f"ari_{tag}_{i}")
                   for i in range(2)]
            outs = [dram.tile([DIM // 2, TOK], f16, addr_space="Shared",
                              name=f"aro_{tag}_{i}") for i in range(2)]
            return ins, outs

        for t in range(NCH):
            # x chunk (fp16); issued early on the sync queue
            x16 = xpool.tile([128, KT, TOK], f16, tag="x", name="x16")
            nc.sync.dma_start(x16[:],
                              XT[t].rearrange("(k p) t2 -> p k t2", p=128))
            # streamed FFN weights for this chunk, split at the k-half
            w1a = [w13r.tile([128, 8, 128], f16, tag="w1a", name=f"w1a{n}",
                             bufs=6) for n in range(HKT)]
            w3a = [w13r.tile([128, 8, 128], f16, tag="w3a", name=f"w3a{n}",
                             bufs=6) for n in range(HKT)]
            w1b = [w13r.tile([128, 8, 128], f16, tag="w1b", name=f"w1b{n}",
                             bufs=3) for n in range(HKT)]
            w3b = [w13r.tile([128, 8, 128], f16, tag="w3b", name=f"w3b{n}",
                             bufs=3) for n in range(HKT)]
            for n in range(HKT):
                nc.sync.dma_start(w3a[n][:], W3[n, :, 0:8])
                nc.sync.dma_start(w1a[n][:], W1[n, :, 0:8])

            # ---- 1. om1 partials = a_sh @ wm rows; AR1 in feature halves
            arin1, arout1 = mk_arbufs(f"a1_{t}")
            rowproj_stage(wmr, a_sh, [(arin1[0], 2), (arin1[1], 2)])
            allreduce(arin1[0][:], arout1[0][:])
            allreduce(arin1[1][:], arout1[1][:])

            # ---- 2. yo partials for prev chunk (fills AR1h0 window); its RS
            # goes on the CC queue between AR1 and AR2.
            if t > 0:
                yin = dram.tile([DIM, TOK], f16, name=f"yin_{t}")
                rowproj_stage(wor, a_sh, [(yin, 4)])
                reduce_scatter(yin, YO[t - 1], f"y{t - 1}")

            # ---- 3. x-side q projection + rope (AR1h0 window)
            qT = qkpool.tile([128, 2, TOK], f16, tag="qT", name="qT")
            qps = colproj2_psums(wqr, x16, psA, "pa", 0, KT)
            rope2(lambda h, ri: qT[ri * 64:(ri + 1) * 64, h, :],
                  qps, cosx, sinx)

            # ---- 4. om1 halves land as they arrive
            om1 = ompool.tile([128, KT, TOK], f16, tag="om", name="om1")
            for half in range(2):
                nc.sync.dma_start(
                    om1[:, half * 8:(half + 1) * 8, :],
                    arout1[half][:].rearrange("(k p) t2 -> p k t2", p=128))

            # ---- 5. FFN up with k-split: k0-7 on om1h0 (spilled to SBUF),
            # k8-15 + resume after om1h1. ssq1 accumulates across.
            sp1 = sppool.tile([128, HKT, TOK], f16, tag="sp1", name="sp1")
            sp3 = sppool.tile([128, HKT, TOK], f16, tag="sp3", name="sp3")
            ssq1 = psN.tile([1, TOK], f32, tag="pn", name="ssq1")
            for k in range(8):
                sq = scr.tile([128, TOK], f16, tag="sq16", name="sq", bufs=1)
                nc.gpsimd.tensor_mul(sq[:], om1[:, k, :], om1[:, k, :])
                mm(ssq1[:], ones16[:], sq[:], k == 0, False)
            for n in range(HKT):
                p3 = psA.tile([128, TOK], f32, tag="pa", name="p3s")
                for k in range(8):
                    mm(p3[:], w3a[n][:, k, :], om1[:, k, :], k == 0, k == 7)
                nc.scalar.copy(sp3[:, n, :], p3[:])
            for n in range(HKT):
                p1 = psA.tile([128, TOK], f32, tag="pa", name="p1s")
                for k in range(8):
                    mm(p1[:], w1a[n][:, k, :], om1[:, k, :], k == 0, k == 7)
                nc.scalar.copy(sp1[:, n, :], p1[:])
            for n in range(HKT):
                nc.sync.dma_start(w1b[n][:], W1[n, :, 8:KT])
                nc.sync.dma_start(w3b[n][:], W3[n, :, 8:KT])
            # second half (waits om1h1): finish ssq then tiles
            for k in range(8, KT):
                sq = scr.tile([128, TOK], f16, tag="sq16", name="sq", bufs=1)
                nc.gpsimd.tensor_mul(sq[:], om1[:, k, :], om1[:, k, :])
                mm(ssq1[:], ones16[:], sq[:], False, k == KT - 1)
            bc1 = bc_from_psum(ssq1, "rstd", f"bc1_{t}")
            g = gpool.tile([128, HKT, TOK], f16, tag="g", name="g")
            for n in range(HKT):
                p1 = psA.tile([128, TOK], f32, tag="pa", name="p1b")
                for k in range(8, KT):
                    mm(p1[:], w1b[n][:, k - 8, :], om1[:, k, :], k == 8,
                       k == KT - 1)
                p3 = psA.tile([128, TOK], f32, tag="pa", name="p3b")
                for k in range(8, KT):
                    mm(p3[:], w3b[n][:, k - 8, :], om1[:, k, :], k == 8,
                       k == KT - 1)
                t1f = scr.tile([128, TOK], f16, tag="tmp1", name="t1f", bufs=2)
                nc.vector.scalar_tensor_tensor(
                    t1f[:], p1[:], 1.0, sp1[:, n, :],
                    op0=AluOp.mult, op1=AluOp.add)
                s1 = scr.tile([128, TOK], f16, tag="s1t", name="s1", bufs=2)
                nc.vector.tensor_mul(s1[:], t1f[:], bc1[:])
                sil = scr.tile([128, TOK], f16, tag="silt", name="sil", bufs=2)
                nc.scalar.activation(sil[:], s1[:], AFT.Silu)
                t3f = scr.tile([128, TOK], f16, tag="tmp1", name="t3f", bufs=2)
                nc.vector.scalar_tensor_tensor(
                    t3f[:], p3[:], 1.0, sp3[:, n, :],
                    op0=AluOp.mult, op1=AluOp.add)
                m_ = scr.tile([128, TOK], f16, tag="s1t", name="m_", bufs=2)
                nc.vector.tensor_mul(m_[:], t3f[:], sil[:])
                nc.gpsimd.tensor_mul(g[:, n, :], m_[:], bc1[:])

            # ---- 6. FFN down + residual; AR2 in feature halves
            arin2, arout2 = mk_arbufs(f"a2_{t}")
            for q in range(4):
                w2p = []
                for jp in range(2):
                    w2t = w2r.tile([128, 2, HKT, 128], f16, tag="w2r",
                                   name="w2t")
                    nc.sync.dma_start(w2t[:, 0], W2[4 * q + 2 * jp])
                    nc.sync.dma_start(w2t[:, 1], W2[4 * q + 2 * jp + 1])
                    w2p.append(w2t)
                stq = stage.tile([128, 4, TOK], f16, tag="stage", name="st2")
                for j in range(4):
                    nf = 4 * q + j
                    p = psA.tile([128, TOK], f32, tag="pa", name="pd")
                    for k in range(HKT):
                        mm(p[:], w2p[j // 2][:, j % 2, k, :], g[:, k, :],
                           k == 0, k == HKT - 1)
                    nc.vector.scalar_tensor_tensor(
                        stq[:, j, :], om1[:, nf, :], 1.0 / NC, p[:],
                        op0=AluOp.mult, op1=AluOp.add)
                dst2 = arin2[0] if q < 2 else arin2[1]
                q2 = q if q < 2 else q - 2
                nc.sync.dma_start(
                    dst2[q2 * 4 * 128:(q2 + 1) * 4 * 128, :]
                    .rearrange("(j p) t -> p j t", p=128),
                    stq[:])
                if q == 1:
                    allreduce(arin2[0][:], arout2[0][:])
            allreduce(arin2[1][:], arout2[1][:])

            # ---- 7. x-side k + v (AR2 windows)
            kall = qkpool.tile([128, 2, B, 2 * MEM], f16, tag="kall",
                               name="kall")
            kps = colproj2_psums(wkr, x16, psA, "pa", 0, KT)
            rope2(lambda h, ri: kall[ri * 64:(ri + 1) * 64, h, :, MEM:],
                  kps, cosx, sinx)
            vx = vpool.tile([128, B, 256], f16, tag="vx", name="vx")
            vproj_tok(x16, wvr, vx, [0, 1, 2, 3], 0, KT)

            # ---- 8. om2 halves; KM/VM prefills on h0, resume after h1
            om2 = ompool.tile([128, KT, TOK], f16, tag="om", name="om2")
            for half in range(2):
                nc.sync.dma_start(
                    om2[:, half * 8:(half + 1) * 8, :],
                    arout2[half][:].rearrange("(k p) t2 -> p k t2", p=128))

            ssq2 = psN.tile([1, TOK], f32, tag="pn", name="ssq2")
            for k in range(8):
                sq = scr.tile([128, TOK], f16, tag="sq16", name="sq2", bufs=1)
                nc.gpsimd.tensor_mul(sq[:], om2[:, k, :], om2[:, k, :])
                mm(ssq2[:], ones16[:], sq[:], k == 0, False)
            kmps = [psF.tile([128, TOK], f32, tag="pf", name=f"km{n}")
                    for n in range(2)]
            colproj2_psums(wkmr, om2, psF, "pf", 0, 8, ps=kmps, stop=False)
            vm = vpool.tile([128, B, 256], f16, tag="vm", name="vm")
            vmps = [psF.tile([128, 256], f32, tag="pf", name=f"vm{b}")
                    for b in range(2)]
            vproj_tok(om2, wvmr, vm, [0, 1], 0, 8, ps=vmps, stop=False)
            # after om2h1:
            for k in range(8, KT):
                sq = scr.tile([128, TOK], f16, tag="sq16", name="sq2", bufs=1)
                nc.gpsimd.tensor_mul(sq[:], om2[:, k, :], om2[:, k, :])
                mm(ssq2[:], ones16[:], sq[:], False, k == KT - 1)
            bc2 = bc_from_psum(ssq2, "rstd", f"bc2_{t}")
            colproj2_psums(wkmr, om2, psF, "pf", 8, KT, ps=kmps, stop=True)
            rope2(lambda h, ri: kall[ri * 64:(ri + 1) * 64, h, :, 0:MEM],
                  kmps, cosm, sinm)
            vproj_tok(om2, wvmr, vm, [0, 1], 8, KT, ps=vmps, stop=True)
            vproj_tok(om2, wvmr, vm, [2, 3], 0, KT)

            # rstd2 per mem-token: transpose bc2 blocks -> [128, B]
            rstd2T = smol.tile([128, B], f32, tag="r2T", name="r2T")
            for b in range(B):
                ptr = psA.tile([128, 128], f32, tag="pa", name="ptr")
                nc.tensor.transpose(ptr[:], bc2[:, b * 128:(b + 1) * 128],
                                    ident32[:])
                nc.vector.tensor_copy(rstd2T[:, b:b + 1], ptr[:, 0:1])

            # ---- 9. attention; k-major scores; denominators accumulate in
            # one [1,TOK] psum; 1/den via broadcast + approx reciprocal.
            aout = aoutp.tile([128, 2, TOK], f16, tag="aout", name=f"aout_{t}")
            for h in range(HPC):
                eT = scr.tile([128, B, 2, MEM], f16, tag="eT", name="eT",
                              bufs=1)
                sT = scr.tile([128, B, 2, MEM], f32, tag="sT", name="sT",
                              bufs=1)
                for b in range(B):
                    ps = psA.tile([128, 2, MEM], f32, tag="pa", name="ps")
                    mm(ps[:, 0, :], kall[:, h, b, 0:MEM],
                       qT[:, h, b * 128:(b + 1) * 128], True, True)
                    mm(ps[:, 1, :], kall[:, h, b, MEM:],
                       qT[:, h, b * 128:(b + 1) * 128], True, True)
                    nc.vector.tensor_scalar_mul(sT[:, b, 0, :], ps[:, 0, :],
                                                rstd2T[:, b:b + 1])
                    nc.vector.tensor_add(sT[:, b, 1, :], ps[:, 1, :],
                                         maskc[:, b, :])
                    nc.scalar.activation(eT[:, b, :, :], sT[:, b, :, :],
                                         AFT.Exp)
                pden = psN.tile([1, TOK], f32, tag="pn", name="pden")
                for b in range(B):
                    mm(pden[:, b * 128:(b + 1) * 128], ones16[:],
                       eT[:, b, 0, :], True, False)
                    mm(pden[:, b * 128:(b + 1) * 128], ones16[:],
                       eT[:, b, 1, :], False, True)
                rb = bc_from_psum(pden, "recip", f"rb{h}_{t}")
                # mem-key rows of e additionally scaled by rstd2 (v folding)
                for b in range(B):
                    nc.gpsimd.tensor_scalar_mul(eT[:, b, 0, :], eT[:, b, 0, :],
                                                rstd2T[:, b:b + 1])
                for b in range(B):
                    po = psA.tile([128, 128], f32, tag="pa", name="po")
                    mm(po[:], vm[:, b, h * 128:(h + 1) * 128],
                       eT[:, b, 0, :], True, False)
                    mm(po[:], vx[:, b, h * 128:(h + 1) * 128],
                       eT[:, b, 1, :], False, True)
                    nc.vector.tensor_mul(aout[:, h, b * 128:(b + 1) * 128],
                                           po[:], rb[:, b * 128:(b + 1) * 128])
            a_sh = aout

        # final yo for last chunk
        yin = dram.tile([DIM, TOK], f16, name="yin_last")
        rowproj_stage(wor, a_sh, [(yin, 4)])
        reduce_scatter(yin, YO[NCH - 1], "ylast")
        es.close()

    nc.compile()
    return nc


def _get_runtime():
    if "nc" not in _RUNTIME:
        _RUNTIME["nc"] = _build()
    return _RUNTIME["nc"]


def _assemble(results):
    out = np.zeros((B, SEQ, DIM), np.float32)
    for c in range(NC):
        yo = np.asarray(results[c]["YO"], np.float32)  # [NCH, FPC, TOK]
        y = yo.reshape(NCH, FPC, B, MEM).transpose(2, 0, 3, 1)
        out[:, :, c * FPC:(c + 1) * FPC] = y.reshape(B, SEQ, FPC)
    return out


def kernel(**inputs):
    from concourse.bass_utils import run_bass_kernel_spmd
    nc = _get_runtime()
    in_maps = _prepare(inputs)
    res = run_bass_kernel_spmd(nc, in_maps, core_ids=list(range(NC)),
                               trace=False)
    return _assemble(res.results)


if __name__ == "__main__":
    _build()
    print("build ok")


# revision 10
# speedup vs baseline: 1.1001x; 1.1001x over previous
"""nn_MemoryAttention TP8 Trainium2 kernel, v3.

8 NeuronCores, T-layout activations [feature, token] (512 token cols =
4 batch x 128). Attention output stays head-sharded (2 heads = 256 features
per core); wm and wo are ROW-parallel on that shard; per-chunk recurrence
needs two AllReduces (om1 = a_sh @ wm, om2 = om1 + FFN) plus a per-chunk
ReduceScatter for yo = a_sh @ wo.

v3 changes vs v2:
- All weights and activations fp16 (incl. wm/wo/aout); f32 only in psum,
  softmax/norm scalar chains.
- x/mem projection weights SBUF-resident (no per-chunk reloads).
- FFN up-projection k-tiles 0-7 computed during the AR1 wire time of
  k-tiles 8-15 ("spill" partial psums to SBUF fp16, resume + add after the
  second AR half lands). Same trick hides AR2 behind KM/VM prefills.
- v projections computed directly in token-partition layout (weight as the
  moving operand), killing the per-chunk PE transposes.
- softmax/rms epilogues: psum-accumulated denominators, broadcast first,
  then reciprocal_approx_fast on the [128, 512] tile (the serial [1,512]
  DVE reciprocal was 3.3us each).
- yo ReduceScatter queued between AR1 and AR2 so it never delays the
  om-recurrence collectives.
"""
import numpy as np

DIM = 2048; NH = 16; HD = 128; MEM = 128; SEQ = 2048; B = 4; HID = 5632
EPS = 1e-5
NC = 8
HPC = NH // NC          # 2 heads per core
FPC = DIM // NC         # 256 features per core
HIDP = 768              # padded per-core FFN hidden (704 -> 768)
NCH = SEQ // MEM        # 16 chunks
TOK = B * MEM           # 512 token columns
KT = DIM // 128         # 16 feature k-tiles
HKT = HIDP // 128       # 6 hidden k-tiles
MASKV = -60.0

_RUNTIME = {}
COLL = True   # False: replace collectives with local DMAs (timing diagnostic)


def _head_perm():
    p = np.concatenate([np.arange(0, HD, 2), np.arange(1, HD, 2)])
    return np.concatenate([h * HD + p for h in range(NH)])


def _slab(w):
    """Column-shard weight [2048, C] -> [C//128, 128, KT, 128] (stationary)."""
    C = w.shape[1]
    return np.ascontiguousarray(
        w.reshape(KT, 128, C // 128, 128).transpose(2, 1, 0, 3))


def _slab_row(w):
    """Row-shard weight [256, 2048] -> [2, 128, 16, 128] (k-major)."""
    return np.ascontiguousarray(w.reshape(2, 128, 16, 128))


def _slab_mov(w):
    """Moving-operand weight [2048, 256] -> [KT, 128, 256]."""
    return np.ascontiguousarray(w.reshape(KT, 128, 256))


def _slab_w2(w2):
    """[768, 2048] -> [16, 128, HKT, 128]."""
    return np.ascontiguousarray(
        w2.reshape(HKT, 128, 16, 128).transpose(2, 1, 0, 3))


def _trunc22(x):
    u = np.ascontiguousarray(x, np.float32).view(np.uint32)
    return ((u + np.uint32(0x200)) & np.uint32(0xFFFFFC00)).view(np.float32)


def _prepare(inputs):
    perm = _head_perm()
    scale = 1.0 / np.sqrt(HD)
    fwv = np.asarray(inputs["ffn_norm_w"], np.float32)
    mwv = np.asarray(inputs["mem_norm_w"], np.float32)
    wq = np.asarray(inputs["wq"])[:, perm] * scale
    wk = np.asarray(inputs["wk"])[:, perm]
    wkm = (mwv[:, None] * np.asarray(inputs["wkm"]))[:, perm]
    wvm = mwv[:, None] * np.asarray(inputs["wvm"])
    wv = np.asarray(inputs["wv"])
    wm = np.asarray(inputs["wm"]); wo = np.asarray(inputs["wo"])
    w1 = np.zeros((DIM, NC * HIDP), np.float32)
    w3 = np.zeros((DIM, NC * HIDP), np.float32)
    w2 = np.zeros((NC * HIDP, DIM), np.float32)
    w1s = fwv[:, None] * np.asarray(inputs["w1"])
    w3s = fwv[:, None] * np.asarray(inputs["w3"])
    for c in range(NC):
        w1[:, c * HIDP:c * HIDP + 704] = w1s[:, c * 704:(c + 1) * 704]
        w3[:, c * HIDP:c * HIDP + 704] = w3s[:, c * 704:(c + 1) * 704]
        w2[c * HIDP:c * HIDP + 704] = np.asarray(inputs["w2"])[c * 704:(c + 1) * 704]
    fc = np.asarray(inputs["freqs_cos"]); fs = np.asarray(inputs["freqs_sin"])
    cos_mem = _trunc22(np.tile(fc[0:MEM].T, (1, B)))      # [64, 512]
    sin_mem = _trunc22(np.tile(fs[0:MEM].T, (1, B)))
    cos_x = _trunc22(np.tile(fc[MEM:2 * MEM].T, (1, B)))
    sin_x = _trunc22(np.tile(fs[MEM:2 * MEM].T, (1, B)))
    # causal mask for chunk-key columns only, transposed: [128 k, 4 b, 128 q]
    mask1 = np.zeros((MEM, MEM), np.float32)
    for i in range(MEM):
        mask1[i, i + 1:] = MASKV
    maskc = np.ascontiguousarray(
        np.broadcast_to(mask1.T[:, None, :], (MEM, B, MEM))).astype(np.float16)
    x = np.asarray(inputs["x"])
    xT = np.ascontiguousarray(
        x.reshape(B, NCH, MEM, DIM).transpose(1, 3, 0, 2)
        .reshape(NCH, DIM, TOK)).astype(np.float16)
    om0 = np.asarray(inputs["origin_mem"])
    omT0 = om0.transpose(2, 0, 1).reshape(DIM, TOK)
    in_maps = []
    for c in range(NC):
        hsl = slice(c * FPC, (c + 1) * FPC)
        hidsl = slice(c * HIDP, (c + 1) * HIDP)
        in_maps.append({
            "WM": _slab_row(wm[hsl, :]).astype(np.float16),
            "WO": _slab_row(wo[hsl, :]).astype(np.float16),
            "WKM": _slab(wkm[:, hsl]).astype(np.float16),
            "WVM": _slab_mov(wvm[:, hsl]).astype(np.float16),
            "WQ": _slab(wq[:, hsl]).astype(np.float16),
            "WK": _slab(wk[:, hsl]).astype(np.float16),
            "WV": _slab_mov(wv[:, hsl]).astype(np.float16),
            "W1": _slab(w1[:, hidsl]).astype(np.float16),
            "W3": _slab(w3[:, hidsl]).astype(np.float16),
            "W2": _slab_w2(w2[hidsl, :]).astype(np.float16),
            "XT": xT,
            "OM0T": omT0[c * FPC:(c + 1) * FPC].reshape(2, 128, TOK)
                    .astype(np.float16),
            "COSM": cos_mem.astype(np.float16), "SINM": sin_mem.astype(np.float16),
            "COSX": cos_x.astype(np.float16), "SINX": sin_x.astype(np.float16),
            "MASKC": maskc,
        })
    return in_maps


def _build():
    import concourse.bacc as bacc
    import concourse.tile as tile
    import concourse.mybir as mybir
    from concourse.masks import make_identity
    from contextlib import ExitStack

    dt = mybir.dt
    AluOp = mybir.AluOpType
    AFT = mybir.ActivationFunctionType
    f32, f32r, f16 = dt.float32, dt.float32r, dt.float16

    nc = bacc.Bacc("TRN2", target_bir_lowering=False, debug=False,
                   num_devices=NC)

    def din(name, shape, dtype=f16):
        return nc.dram_tensor(name, shape, dtype, kind="ExternalInput")

    WM = din("WM", [2, 128, KT, 128]); WO = din("WO", [2, 128, KT, 128])
    WKM = din("WKM", [2, 128, KT, 128])
    WVM = din("WVM", [KT, 128, 256])
    WQ = din("WQ", [2, 128, KT, 128]); WK = din("WK", [2, 128, KT, 128])
    WV = din("WV", [KT, 128, 256])
    W1 = din("W1", [HKT, 128, KT, 128])
    W3 = din("W3", [HKT, 128, KT, 128])
    W2 = din("W2", [KT, 128, HKT, 128])
    XT = din("XT", [NCH, DIM, TOK])
    OM0T = din("OM0T", [2, 128, TOK])
    COSM = din("COSM", [64, TOK]); SINM = din("SINM", [64, TOK])
    COSX = din("COSX", [64, TOK]); SINX = din("SINX", [64, TOK])
    MASKC = din("MASKC", [MEM, B, MEM])
    YO = nc.dram_tensor("YO", [NCH, FPC, TOK], f16, kind="ExternalOutput")

    rg = [list(range(NC))]

    with tile.TileContext(nc) as tc:
        es = ExitStack()
        const = es.enter_context(tc.tile_pool(name="const", bufs=1))
        wres = es.enter_context(tc.tile_pool(name="wres", bufs=1))
        w13r = es.enter_context(tc.tile_pool(name="w13r", bufs=4))
        w2r = es.enter_context(tc.tile_pool(name="w2r", bufs=2))
        ompool = es.enter_context(tc.tile_pool(name="ompool", bufs=1))
        xpool = es.enter_context(tc.tile_pool(name="xpool", bufs=1))
        gpool = es.enter_context(tc.tile_pool(name="gpool", bufs=1))
        sppool = es.enter_context(tc.tile_pool(name="sppool", bufs=1))
        qkpool = es.enter_context(tc.tile_pool(name="qkpool", bufs=1))
        vpool = es.enter_context(tc.tile_pool(name="vpool", bufs=1))
        stage = es.enter_context(tc.tile_pool(name="stage", bufs=2))
        scr = es.enter_context(tc.tile_pool(name="scr", bufs=2))
        smol = es.enter_context(tc.tile_pool(name="smol", bufs=2))
        aoutp = es.enter_context(tc.tile_pool(name="aoutp", bufs=2))
        dram = es.enter_context(tc.tile_pool(name="dram", bufs=1, space="DRAM"))
        psA = es.enter_context(tc.tile_pool(name="psA", bufs=3, space="PSUM"))
        psF = es.enter_context(tc.tile_pool(name="psF", bufs=4, space="PSUM"))
        psN = es.enter_context(tc.tile_pool(name="psN", bufs=1, space="PSUM"))

        # ---- constants
        cosm = const.tile([64, TOK], f16); nc.sync.dma_start(cosm[:], COSM[:])
        sinm = const.tile([64, TOK], f16); nc.sync.dma_start(sinm[:], SINM[:])
        cosx = const.tile([64, TOK], f16); nc.sync.dma_start(cosx[:], COSX[:])
        sinx = const.tile([64, TOK], f16); nc.sync.dma_start(sinx[:], SINX[:])
        maskc = const.tile([MEM, B, MEM], f16)
        nc.sync.dma_start(maskc[:], MASKC[:])
        scratch32 = const.tile([128, 128], f32)
        nc.vector.memset(scratch32[:], 1.0)
        ones16 = const.tile([128, 1], f16)
        nc.vector.tensor_copy(ones16[:], scratch32[:, 0:1])
        onesrow = const.tile([1, 128], f32r)
        nc.vector.tensor_copy(onesrow[:], scratch32[0:1, :])
        ident32 = const.tile([128, 128], f32)
        make_identity(nc, ident32)
        epst = const.tile([128, 1], f32)
        nc.vector.memset(epst[:], EPS)

        # ---- resident weights
        def wload(Wd, n_slabs, tag):
            w = wres.tile([128, n_slabs, KT, 128], f16, tag=tag, name=tag)
            for n in range(n_slabs):
                nc.sync.dma_start(w[:, n], Wd[n])
            return w

        wmr = wload(WM, 2, "wmr"); wor = wload(WO, 2, "wor")
        wqr = wload(WQ, 2, "wqr"); wkr = wload(WK, 2, "wkr")
        wkmr = wload(WKM, 2, "wkmr")
        wvr = wres.tile([128, KT, 256], f16, tag="wvr", name="wvr")
        nc.sync.dma_start(wvr[:], WV[:].rearrange("k p f -> p k f"))
        wvmr = wres.tile([128, KT, 256], f16, tag="wvmr", name="wvmr")
        nc.sync.dma_start(wvmr[:], WVM[:].rearrange("k p f -> p k f"))

        a_sh = aoutp.tile([128, 2, TOK], f16, tag="aout", name="a_init")
        nc.sync.dma_start(a_sh[:], OM0T[:].rearrange("k p t -> p k t"))

        def mm(p, lhsT, rhs, start, stop):
            nc.tensor.matmul(p, lhsT, rhs, start=start, stop=stop)

        def rowproj_stage(Wr, a_tile, writers):
            """Row-parallel 256->2048 proj of a_tile [128,2,TOK] fp16; psum
            quartets staged fp16; writers[qi] gives the DRAM dst AP for
            quartet qi."""
            for qi, wfn in enumerate(writers):
                stq = stage.tile([128, 4, TOK], f16, tag="stage", name="stq")
                for j in range(4):
                    n = 4 * qi + j
                    p = psA.tile([128, TOK], f32, tag="pa", name="pn")
                    mm(p[:], Wr[:, 0, n, :], a_tile[:, 0, :], True, False)
                    mm(p[:], Wr[:, 1, n, :], a_tile[:, 1, :], False, True)
                    if j % 2 == 0:
                        nc.scalar.copy(stq[:, j, :], p[:])
                    else:
                        nc.vector.tensor_copy(stq[:, j, :], p[:])
                nc.sync.dma_start(wfn(), stq[:])

        def allreduce(in_d, out_d):
            if COLL:
                nc.gpsimd.collective_compute(
                    "AllReduce", AluOp.add, replica_groups=rg,
                    ins=[in_d[:].opt()], outs=[out_d[:].opt()])
            else:
                nc.sync.dma_start(out_d[:], in_d[:])

        def reduce_scatter(in_d, out_ap, tag):
            rsout = dram.tile([FPC, TOK], f16, name=f"rsout_{tag}")
            if COLL:
                nc.gpsimd.collective_compute(
                    "ReduceScatter", AluOp.add, replica_groups=rg,
                    ins=[in_d[:].opt()], outs=[rsout[:].opt()])
            else:
                nc.sync.dma_start(rsout[:], in_d[0:FPC, :])
            nc.scalar.dma_start(out_ap, rsout[:])

        def bc_from_psum(ssq_ps, kind, name):
            """[1,TOK] psum -> broadcast [128,TOK] -> rstd (Sqrt+recip) or
            plain reciprocal; returns [128,TOK] f32 SBUF tile."""
            r = smol.tile([1, TOK], f32r, tag="smr", name=f"r_{name}", bufs=2)
            nc.vector.tensor_copy(r[:], ssq_ps[:])
            pbc = psF.tile([128, TOK], f32, tag="pf", name=f"pbc_{name}")
            mm(pbc[:], onesrow[:], r[:], True, True)
            bc = scr.tile([128, TOK], f32, tag=f"bc_{kind}", name=f"bc_{name}",
                          bufs=1 if kind == "rstd" else 2)
            if kind == "rstd":
                sq = scr.tile([128, TOK], f32, tag="sqt", name=f"sq_{name}",
                              bufs=1)
                nc.scalar.activation(sq[:], pbc[:], AFT.Sqrt,
                                     bias=epst[:], scale=1.0 / DIM)
                nc.vector.reciprocal_approx_fast(bc[:], sq[:])
            else:
                nc.vector.reciprocal_approx_fast(bc[:], pbc[:])
            return bc

        def rope2(dst_fn, src01, cosT, sinT):
            """src01: two psum tiles [128, TOK]; dst_fn(h, ri) -> fp16 AP."""
            for h in range(2):
                ph = src01[h]
                r, i = ph[0:64, :], ph[64:128, :]
                t1 = scr.tile([64, TOK], f32, tag="t1", name="t1", bufs=2)
                t2 = scr.tile([64, TOK], f32, tag="t1", name="t2", bufs=2)
                nc.vector.tensor_mul(t1[:], r, cosT[:])
                nc.vector.tensor_mul(t2[:], i, sinT[:])
                nc.vector.tensor_sub(dst_fn(h, 0), t1[:], t2[:])
                t3 = scr.tile([64, TOK], f32, tag="t1", name="t3", bufs=2)
                t4 = scr.tile([64, TOK], f32, tag="t1", name="t4", bufs=2)
                nc.vector.tensor_mul(t3[:], r, sinT[:])
                nc.vector.tensor_mul(t4[:], i, cosT[:])
                nc.vector.tensor_add(dst_fn(h, 1), t3[:], t4[:])

        def colproj2_psums(Wr, rhs_tile, pool, tag, k0, k1, ps=None,
                           stop=True):
            """2048 -> 256 col proj over k in [k0,k1); returns 2 psum tiles."""
            outs = []
            for n in range(2):
                if ps is None:
                    p = pool.tile([128, TOK], f32, tag=tag, name=f"pp{n}")
                else:
                    p = ps[n]
                for k in range(k0, k1):
                    mm(p[:], Wr[:, n, k, :], rhs_tile[:, k, :], k == k0 and k0 == 0,
                       stop and k == k1 - 1)
                outs.append(p)
            return outs

        def vproj_tok(lhs_tile, Wr, v, bs, k0, k1, ps=None, stop=True):
            """v[tok, b, 256] += x[k-tile, b-block]^T @ W[k-tile] over k."""
            outs = []
            for bi, b in enumerate(bs):
                if ps is None:
                    p = psA.tile([128, 256], f32, tag="pa", name=f"pv{b}")
                else:
                    p = ps[bi]
                for k in range(k0, k1):
                    mm(p[:], lhs_tile[:, k, b * 128:(b + 1) * 128],
                       Wr[:, k, :], k == k0 and k0 == 0, stop and k == k1 - 1)
                outs.append(p)
                if stop:
                    nc.scalar.copy(v[:, b, :], p[:])
            return outs

        # DRAM comm buffers
        def mk_arbufs(tag):
            ins = [dram.tile([128, 8, TOK], f16, name=f"ari_{tag}_{i}")
                   for i in range(2)]
            outs = [dram.tile([128, 8, TOK], f16, addr_space="Shared",
                              name=f"aro_{tag}_{i}") for i in range(2)]
            return ins, outs

        for t in range(NCH):
            # x chunk (fp16); issued early on the sync queue
            x16 = xpool.tile([128, KT, TOK], f16, tag="x", name="x16")
            nc.sync.dma_start(x16[:],
                              XT[t].rearrange("(k p) t2 -> p k t2", p=128))
            # streamed FFN weights for this chunk, split at the k-half
            w1a = [w13r.tile([128, 8, 128], f16, tag="w1a", name=f"w1a{n}",
                             bufs=6) for n in range(HKT)]
            w3a = [w13r.tile([128, 8, 128], f16, tag="w3a", name=f"w3a{n}",
                             bufs=6) for n in range(HKT)]
            w1b = [w13r.tile([128, 8, 128], f16, tag="w1b", name=f"w1b{n}",
                             bufs=3) for n in range(HKT)]
            w3b = [w13r.tile([128, 8, 128], f16, tag="w3b", name=f"w3b{n}",
                             bufs=3) for n in range(HKT)]
            for n in range(HKT):
                nc.sync.dma_start(w3a[n][:], W3[n, :, 0:8])
                nc.sync.dma_start(w1a[n][:], W1[n, :, 0:8])
            for n in range(HKT):
                nc.sync.dma_start(w1b[n][:], W1[n, :, 8:KT])
                nc.sync.dma_start(w3b[n][:], W3[n, :, 8:KT])

            # ---- 1. om1 partials = a_sh @ wm rows; AR1 in feature halves
            arin1, arout1 = mk_arbufs(f"a1_{t}")
            rowproj_stage(wmr, a_sh,
                          [lambda q=q: arin1[q // 2][:, (q % 2) * 4:
                                                     (q % 2) * 4 + 4, :]
                           for q in range(4)])
            allreduce(arin1[0][:], arout1[0][:])
            allreduce(arin1[1][:], arout1[1][:])

            # ---- 2. yo partials for prev chunk (fills AR1h0 window); its RS
            # goes on the CC queue between AR1 and AR2.
            if t > 0:
                yin = dram.tile([DIM, TOK], f16, name=f"yin_{t}")
                rowproj_stage(wor, a_sh,
                              [lambda q=q: yin[q * 512:(q + 1) * 512, :]
                               .rearrange("(j p) t2 -> p j t2", p=128)
                               for q in range(4)])
                reduce_scatter(yin, YO[t - 1], f"y{t - 1}")

            # ---- 3. x-side q projection + rope (AR1h0 window)
            qT = qkpool.tile([128, 2, TOK], f16, tag="qT", name="qT")
            qps = colproj2_psums(wqr, x16, psA, "pa", 0, KT)
            rope2(lambda h, ri: qT[ri * 64:(ri + 1) * 64, h, :],
                  qps, cosx, sinx)

            # ---- 4. om1 halves land as they arrive
            om1 = ompool.tile([128, KT, TOK], f16, tag="om", name="om1")
            for half in range(2):
                nc.sync.dma_start(om1[:, half * 8:(half + 1) * 8, :],
                                  arout1[half][:])

            # ---- 5. FFN up with k-split: k0-7 on om1h0 (spilled to SBUF),
            # k8-15 + resume after om1h1. ssq1 accumulates across.
            sp1 = sppool.tile([128, HKT, TOK], f16, tag="sp1", name="sp1")
            sp3 = sppool.tile([128, HKT, TOK], f16, tag="sp3", name="sp3")
            ssq1 = psN.tile([1, TOK], f32, tag="pn", name="ssq1")
            for k in range(8):
                sq = scr.tile([128, TOK], f16, tag="sq16", name="sq", bufs=3)
                nc.gpsimd.tensor_mul(sq[:], om1[:, k, :], om1[:, k, :])
                mm(ssq1[:], ones16[:], sq[:], k == 0, False)
            for n in range(HKT):
                p3 = psA.tile([128, TOK], f32, tag="pa", name="p3s")
                for k in range(8):
                    mm(p3[:], w3a[n][:, k, :], om1[:, k, :], k == 0, k == 7)
                nc.scalar.copy(sp3[:, n, :], p3[:])
            for n in range(HKT):
                p1 = psA.tile([128, TOK], f32, tag="pa", name="p1s")
                for k in range(8):
                    mm(p1[:], w1a[n][:, k, :], om1[:, k, :], k == 0, k == 7)
                nc.scalar.copy(sp1[:, n, :], p1[:])
            for n in range(HKT):
                nc.sync.dma_start(w1b[n][:], W1[n, :, 8:KT])
                nc.sync.dma_start(w3b[n][:], W3[n, :, 8:KT])
            # second half (waits om1h1): finish ssq then tiles
            for k in range(8, KT):
                sq = scr.tile([128, TOK], f16, tag="sq16", name="sq", bufs=3)
                nc.gpsimd.tensor_mul(sq[:], om1[:, k, :], om1[:, k, :])
                mm(ssq1[:], ones16[:], sq[:], False, k == KT - 1)
            bc1 = bc_from_psum(ssq1, "rstd", f"bc1_{t}")
            g = gpool.tile([128, HKT, TOK], f16, tag="g", name="g")
            for n in range(HKT):
                p1 = psA.tile([128, TOK], f32, tag="pa", name="p1b")
                for k in range(8, KT):
                    mm(p1[:], w1b[n][:, k - 8, :], om1[:, k, :], k == 8,
                       k == KT - 1)
                p3 = psA.tile([128, TOK], f32, tag="pa", name="p3b")
                for k in range(8, KT):
                    mm(p3[:], w3b[n][:, k - 8, :], om1[:, k, :], k == 8,
                       k == KT - 1)
                t1f = scr.tile([128, TOK], f16, tag="tmp1", name="t1f", bufs=2)
                nc.vector.scalar_tensor_tensor(
                    t1f[:], p1[:], 1.0, sp1[:, n, :],
                    op0=AluOp.mult, op1=AluOp.add)
                s1 = scr.tile([128, TOK], f16, tag="s1t", name="s1", bufs=2)
                nc.vector.tensor_mul(s1[:], t1f[:], bc1[:])
                sil = scr.tile([128, TOK], f16, tag="silt", name="sil", bufs=2)
                nc.scalar.activation(sil[:], s1[:], AFT.Silu)
                t3f = scr.tile([128, TOK], f16, tag="tmp1", name="t3f", bufs=2)
                nc.vector.scalar_tensor_tensor(
                    t3f[:], p3[:], 1.0, sp3[:, n, :],
                    op0=AluOp.mult, op1=AluOp.add)
                m_ = scr.tile([128, TOK], f16, tag="s1t", name="m_", bufs=2)
                nc.vector.tensor_mul(m_[:], t3f[:], sil[:])
                nc.gpsimd.tensor_mul(g[:, n, :], m_[:], bc1[:])

            # ---- 6. FFN down + residual; AR2 in feature halves
            arin2, arout2 = mk_arbufs(f"a2_{t}")
            for q in range(4):
                w2p = []
                for jp in range(2):
                    w2t = w2r.tile([128, 2, HKT, 128], f16, tag="w2r",
                                   name="w2t")
                    nc.sync.dma_start(w2t[:, 0], W2[4 * q + 2 * jp])
                    nc.sync.dma_start(w2t[:, 1], W2[4 * q + 2 * jp + 1])
                    w2p.append(w2t)
                stq = stage.tile([128, 4, TOK], f16, tag="stage", name="st2")
                for j in range(4):
                    nf = 4 * q + j
                    p = psA.tile([128, TOK], f32, tag="pa", name="pd")
                    for k in range(HKT):
                        mm(p[:], w2p[j // 2][:, j % 2, k, :], g[:, k, :],
                           k == 0, k == HKT - 1)
                    nc.vector.scalar_tensor_tensor(
                        stq[:, j, :], om1[:, nf, :], 1.0 / NC, p[:],
                        op0=AluOp.mult, op1=AluOp.add)
                dst2 = arin2[0] if q < 2 else arin2[1]
                q2 = q if q < 2 else q - 2
                nc.sync.dma_start(dst2[:, q2 * 4:(q2 + 1) * 4, :], stq[:])
                if q == 1:
                    allreduce(arin2[0][:], arout2[0][:])
            allreduce(arin2[1][:], arout2[1][:])

            # ---- 7. x-side k + v (AR2 windows)
            kall = qkpool.tile([128, 2, B, 2 * MEM], f16, tag="kall",
                               name="kall")
            kps = colproj2_psums(wkr, x16, psA, "pa", 0, KT)
            rope2(lambda h, ri: kall[ri * 64:(ri + 1) * 64, h, :, MEM:],
                  kps, cosx, sinx)
            vx = vpool.tile([128, B, 256], f16, tag="vx", name="vx")
            vproj_tok(x16, wvr, vx, [0, 1, 2, 3], 0, KT)

            # ---- 8. om2 halves; KM/VM prefills on h0, resume after h1
            om2 = ompool.tile([128, KT, TOK], f16, tag="om", name="om2")
            for half in range(2):
                nc.sync.dma_start(om2[:, half * 8:(half + 1) * 8, :],
                                  arout2[half][:])

            ssq2 = psN.tile([1, TOK], f32, tag="pn", name="ssq2")
            for k in range(8):
                sq = scr.tile([128, TOK], f16, tag="sq16", name="sq2", bufs=3)
                nc.gpsimd.tensor_mul(sq[:], om2[:, k, :], om2[:, k, :])
                mm(ssq2[:], ones16[:], sq[:], k == 0, False)
            kmps = [psF.tile([128, TOK], f32, tag="pf", name=f"km{n}")
                    for n in range(2)]
            colproj2_psums(wkmr, om2, psF, "pf", 0, 8, ps=kmps, stop=False)
            vm = vpool.tile([128, B, 256], f16, tag="vm", name="vm")
            vmps = [psF.tile([128, 256], f32, tag="pf", name=f"vm{b}")
                    for b in range(2)]
            vproj_tok(om2, wvmr, vm, [0, 1], 0, 8, ps=vmps, stop=False)
            # after om2h1:
            for k in range(8, KT):
                sq = scr.tile([128, TOK], f16, tag="sq16", name="sq2", bufs=3)
                nc.gpsimd.tensor_mul(sq[:], om2[:, k, :], om2[:, k, :])
                mm(ssq2[:], ones16[:], sq[:], False, k == KT - 1)
            bc2 = bc_from_psum(ssq2, "rstd", f"bc2_{t}")
            colproj2_psums(wkmr, om2, psF, "pf", 8, KT, ps=kmps, stop=True)
            rope2(lambda h, ri: kall[ri * 64:(ri + 1) * 64, h, :, 0:MEM],
                  kmps, cosm, sinm)
            vproj_tok(om2, wvmr, vm, [0, 1], 8, KT, ps=vmps, stop=True)
            vproj_tok(om2, wvmr, vm, [2, 3], 0, KT)

            # rstd2 per mem-token: transpose bc2 blocks -> [128, B]
            rstd2T = smol.tile([128, B], f32, tag="r2T", name="r2T")
            for b in range(B):
                ptr = psA.tile([128, 128], f32, tag="pa", name="ptr")
                nc.tensor.transpose(ptr[:], bc2[:, b * 128:(b + 1) * 128],
                                    ident32[:])
                nc.vector.tensor_copy(rstd2T[:, b:b + 1], ptr[:, 0:1])

            # ---- 9. attention; k-major scores; denominators accumulate in
            # one [1,TOK] psum; 1/den via broadcast + approx reciprocal.
            aout = aoutp.tile([128, 2, TOK], f16, tag="aout", name=f"aout_{t}")
            for h in range(HPC):
                eT = scr.tile([128, B, 2, MEM], f16, tag="eT", name="eT",
                              bufs=1)
                for b in range(B):
                    ps = psA.tile([128, 2, MEM], f32, tag="pa", name="ps")
                    mm(ps[:, 0, :], kall[:, h, b, 0:MEM],
                       qT[:, h, b * 128:(b + 1) * 128], True, True)
                    mm(ps[:, 1, :], kall[:, h, b, MEM:],
                       qT[:, h, b * 128:(b + 1) * 128], True, True)
                    sT = scr.tile([128, 2, MEM], f32, tag="sT", name="sT",
                                  bufs=2)
                    nc.vector.tensor_scalar_mul(sT[:, 0, :], ps[:, 0, :],
                                                rstd2T[:, b:b + 1])
                    nc.vector.tensor_add(sT[:, 1, :], ps[:, 1, :],
                                         maskc[:, b, :])
                    nc.scalar.activation(eT[:, b, :, :], sT[:, :, :],
                                         AFT.Exp)
                pden = psN.tile([1, TOK], f32, tag="pn", name="pden")
                for b in range(B):
                    mm(pden[:, b * 128:(b + 1) * 128], ones16[:],
                       eT[:, b, 0, :], True, False)
                    mm(pden[:, b * 128:(b + 1) * 128], ones16[:],
                       eT[:, b, 1, :], False, True)
                rb = bc_from_psum(pden, "recip", f"rb{h}_{t}")
                # mem-key rows of e additionally scaled by rstd2 (v folding)
                for b in range(B):
                    nc.gpsimd.tensor_scalar_mul(eT[:, b, 0, :], eT[:, b, 0, :],
                                                rstd2T[:, b:b + 1])
                for b in range(B):
                    po = psA.tile([128, 128], f32, tag="pa", name="po")
                    mm(po[:], vm[:, b, h * 128:(h + 1) * 128],
                       eT[:, b, 0, :], True, False)
                    mm(po[:], vx[:, b, h * 128:(h + 1) * 128],
                       eT[:, b, 1, :], False, True)
                    nc.vector.tensor_mul(aout[:, h, b * 128:(b + 1) * 128],
                                           po[:], rb[:, b * 128:(b + 1) * 128])
            a_sh = aout

        # final yo for last chunk
        yin = dram.tile([DIM, TOK], f16, name="yin_last")
        rowproj_stage(wor, a_sh,
                      [lambda q=q: yin[q * 512:(q + 1) * 512, :]
                       .rearrange("(j p) t2 -> p j t2", p=128)
                       for q in range(4)])
        reduce_scatter(yin, YO[NCH - 1], "ylast")
        es.close()

    nc.compile()
    return nc


def _get_runtime():
    if "nc" not in _RUNTIME:
        _RUNTIME["nc"] = _build()
    return _RUNTIME["nc"]


def _assemble(results):
    out = np.zeros((B, SEQ, DIM), np.float32)
    for c in range(NC):
        yo = np.asarray(results[c]["YO"], np.float32)  # [NCH, FPC, TOK]
        y = yo.reshape(NCH, FPC, B, MEM).transpose(2, 0, 3, 1)
        out[:, :, c * FPC:(c + 1) * FPC] = y.reshape(B, SEQ, FPC)
    return out


def kernel(**inputs):
    from concourse.bass_utils import run_bass_kernel_spmd
    nc = _get_runtime()
    in_maps = _prepare(inputs)
    res = run_bass_kernel_spmd(nc, in_maps, core_ids=list(range(NC)),
                               trace=False)
    return _assemble(res.results)


if __name__ == "__main__":
    _build()
    print("build ok")


# revision 13
# speedup vs baseline: 1.1095x; 1.0086x over previous
"""nn_MemoryAttention TP8 Trainium2 kernel, v3.

8 NeuronCores, T-layout activations [feature, token] (512 token cols =
4 batch x 128). Attention output stays head-sharded (2 heads = 256 features
per core); wm and wo are ROW-parallel on that shard; per-chunk recurrence
needs two AllReduces (om1 = a_sh @ wm, om2 = om1 + FFN) plus a per-chunk
ReduceScatter for yo = a_sh @ wo.

v3 changes vs v2:
- All weights and activations fp16 (incl. wm/wo/aout); f32 only in psum,
  softmax/norm scalar chains.
- x/mem projection weights SBUF-resident (no per-chunk reloads).
- FFN up-projection k-tiles 0-7 computed during the AR1 wire time of
  k-tiles 8-15 ("spill" partial psums to SBUF fp16, resume + add after the
  second AR half lands). Same trick hides AR2 behind KM/VM prefills.
- v projections computed directly in token-partition layout (weight as the
  moving operand), killing the per-chunk PE transposes.
- softmax/rms epilogues: psum-accumulated denominators, broadcast first,
  then reciprocal_approx_fast on the [128, 512] tile (the serial [1,512]
  DVE reciprocal was 3.3us each).
- yo ReduceScatter queued between AR1 and AR2 so it never delays the
  om-recurrence collectives.
"""
import numpy as np

DIM = 2048; NH = 16; HD = 128; MEM = 128; SEQ = 2048; B = 4; HID = 5632
EPS = 1e-5
NC = 8
HPC = NH // NC          # 2 heads per core
FPC = DIM // NC         # 256 features per core
HIDP = 768              # padded per-core FFN hidden (704 -> 768)
NCH = SEQ // MEM        # 16 chunks
TOK = B * MEM           # 512 token columns
KT = DIM // 128         # 16 feature k-tiles
HKT = HIDP // 128       # 6 hidden k-tiles
MASKV = -60.0

_RUNTIME = {}
COLL = True   # False: replace collectives with local DMAs (timing diagnostic)


def _head_perm():
    p = np.concatenate([np.arange(0, HD, 2), np.arange(1, HD, 2)])
    return np.concatenate([h * HD + p for h in range(NH)])


def _slab(w):
    """Column-shard weight [2048, C] -> [C//128, 128, KT, 128] (stationary)."""
    C = w.shape[1]
    return np.ascontiguousarray(
        w.reshape(KT, 128, C // 128, 128).transpose(2, 1, 0, 3))


def _slab_row(w):
    """Row-shard weight [256, 2048] -> [2, 128, 16, 128] (k-major)."""
    return np.ascontiguousarray(w.reshape(2, 128, 16, 128))


def _slab_mov(w):
    """Moving-operand weight [2048, 256] -> [KT, 128, 256]."""
    return np.ascontiguousarray(w.reshape(KT, 128, 256))


def _slab_w2(w2):
    """[768, 2048] -> [16, 128, HKT, 128]."""
    return np.ascontiguousarray(
        w2.reshape(HKT, 128, 16, 128).transpose(2, 1, 0, 3))


def _trunc22(x):
    u = np.ascontiguousarray(x, np.float32).view(np.uint32)
    return ((u + np.uint32(0x200)) & np.uint32(0xFFFFFC00)).view(np.float32)


def _prepare(inputs):
    perm = _head_perm()
    scale = 1.0 / np.sqrt(HD)
    fwv = np.asarray(inputs["ffn_norm_w"], np.float32)
    mwv = np.asarray(inputs["mem_norm_w"], np.float32)
    wq = np.asarray(inputs["wq"])[:, perm] * scale
    wk = np.asarray(inputs["wk"])[:, perm]
    wkm = (mwv[:, None] * np.asarray(inputs["wkm"]))[:, perm]
    wvm = mwv[:, None] * np.asarray(inputs["wvm"])
    wv = np.asarray(inputs["wv"])
    wm = np.asarray(inputs["wm"]); wo = np.asarray(inputs["wo"])
    w1 = np.zeros((DIM, NC * HIDP), np.float32)
    w3 = np.zeros((DIM, NC * HIDP), np.float32)
    w2 = np.zeros((NC * HIDP, DIM), np.float32)
    w1s = fwv[:, None] * np.asarray(inputs["w1"])
    w3s = fwv[:, None] * np.asarray(inputs["w3"])
    for c in range(NC):
        w1[:, c * HIDP:c * HIDP + 704] = w1s[:, c * 704:(c + 1) * 704]
        w3[:, c * HIDP:c * HIDP + 704] = w3s[:, c * 704:(c + 1) * 704]
        w2[c * HIDP:c * HIDP + 704] = np.asarray(inputs["w2"])[c * 704:(c + 1) * 704]
    fc = np.asarray(inputs["freqs_cos"]); fs = np.asarray(inputs["freqs_sin"])
    cos_mem = _trunc22(np.tile(fc[0:MEM].T, (1, B)))      # [64, 512]
    sin_mem = _trunc22(np.tile(fs[0:MEM].T, (1, B)))
    cos_x = _trunc22(np.tile(fc[MEM:2 * MEM].T, (1, B)))
    sin_x = _trunc22(np.tile(fs[MEM:2 * MEM].T, (1, B)))
    # causal mask for chunk-key columns only, transposed: [128 k, 4 b, 128 q]
    mask1 = np.zeros((MEM, MEM), np.float32)
    for i in range(MEM):
        mask1[i, i + 1:] = MASKV
    maskc = np.ascontiguousarray(
        np.broadcast_to(mask1.T[:, None, :], (MEM, B, MEM))).astype(np.float16)
    x = np.asarray(inputs["x"])
    xT = np.ascontiguousarray(
        x.reshape(B, NCH, MEM, DIM).transpose(1, 3, 0, 2)
        .reshape(NCH, DIM, TOK)).astype(np.float16)
    om0 = np.asarray(inputs["origin_mem"])
    omT0 = om0.transpose(2, 0, 1).reshape(DIM, TOK)
    in_maps = []
    for c in range(NC):
        hsl = slice(c * FPC, (c + 1) * FPC)
        hidsl = slice(c * HIDP, (c + 1) * HIDP)
        in_maps.append({
            "WM": _slab_row(wm[hsl, :]).astype(np.float16),
            "WO": _slab_row(wo[hsl, :]).astype(np.float16),
            "WKM": _slab(wkm[:, hsl]).astype(np.float16),
            "WVM": _slab_mov(wvm[:, hsl]).astype(np.float16),
            "WQ": _slab(wq[:, hsl]).astype(np.float16),
            "WK": _slab(wk[:, hsl]).astype(np.float16),
            "WV": _slab_mov(wv[:, hsl]).astype(np.float16),
            "W1": _slab(w1[:, hidsl]).astype(np.float16),
            "W3": _slab(w3[:, hidsl]).astype(np.float16),
            "W2": _slab_w2(w2[hidsl, :]).astype(np.float16),
            "XT": xT,
            "OM0T": omT0[c * FPC:(c + 1) * FPC].reshape(2, 128, TOK)
                    .astype(np.float16),
            "COSM": cos_mem.astype(np.float16), "SINM": sin_mem.astype(np.float16),
            "COSX": cos_x.astype(np.float16), "SINX": sin_x.astype(np.float16),
            "MASKC": maskc,
        })
    return in_maps


def _build():
    import concourse.bacc as bacc
    import concourse.tile as tile
    import concourse.mybir as mybir
    from concourse.masks import make_identity
    from contextlib import ExitStack

    dt = mybir.dt
    AluOp = mybir.AluOpType
    AFT = mybir.ActivationFunctionType
    f32, f32r, f16 = dt.float32, dt.float32r, dt.float16

    nc = bacc.Bacc("TRN2", target_bir_lowering=False, debug=False,
                   num_devices=NC)

    def din(name, shape, dtype=f16):
        return nc.dram_tensor(name, shape, dtype, kind="ExternalInput")

    WM = din("WM", [2, 128, KT, 128]); WO = din("WO", [2, 128, KT, 128])
    WKM = din("WKM", [2, 128, KT, 128])
    WVM = din("WVM", [KT, 128, 256])
    WQ = din("WQ", [2, 128, KT, 128]); WK = din("WK", [2, 128, KT, 128])
    WV = din("WV", [KT, 128, 256])
    W1 = din("W1", [HKT, 128, KT, 128])
    W3 = din("W3", [HKT, 128, KT, 128])
    W2 = din("W2", [KT, 128, HKT, 128])
    XT = din("XT", [NCH, DIM, TOK])
    OM0T = din("OM0T", [2, 128, TOK])
    COSM = din("COSM", [64, TOK]); SINM = din("SINM", [64, TOK])
    COSX = din("COSX", [64, TOK]); SINX = din("SINX", [64, TOK])
    MASKC = din("MASKC", [MEM, B, MEM])
    YO = nc.dram_tensor("YO", [NCH, FPC, TOK], f16, kind="ExternalOutput")

    rg = [list(range(NC))]

    with tile.TileContext(nc) as tc:
        es = ExitStack()
        const = es.enter_context(tc.tile_pool(name="const", bufs=1))
        wres = es.enter_context(tc.tile_pool(name="wres", bufs=1))
        w13r = es.enter_context(tc.tile_pool(name="w13r", bufs=4))
        w2r = es.enter_context(tc.tile_pool(name="w2r", bufs=2))
        ompool = es.enter_context(tc.tile_pool(name="ompool", bufs=1))
        xpool = es.enter_context(tc.tile_pool(name="xpool", bufs=1))
        gpool = es.enter_context(tc.tile_pool(name="gpool", bufs=1))
        qkpool = es.enter_context(tc.tile_pool(name="qkpool", bufs=1))
        vpool = es.enter_context(tc.tile_pool(name="vpool", bufs=1))
        stage = es.enter_context(tc.tile_pool(name="stage", bufs=3))
        scr = es.enter_context(tc.tile_pool(name="scr", bufs=2))
        smol = es.enter_context(tc.tile_pool(name="smol", bufs=2))
        aoutp = es.enter_context(tc.tile_pool(name="aoutp", bufs=2))
        dram = es.enter_context(tc.tile_pool(name="dram", bufs=1, space="DRAM"))
        psA = es.enter_context(tc.tile_pool(name="psA", bufs=3, space="PSUM"))
        psF = es.enter_context(tc.tile_pool(name="psF", bufs=4, space="PSUM"))
        psN = es.enter_context(tc.tile_pool(name="psN", bufs=1, space="PSUM"))

        # ---- constants
        cosm = const.tile([64, TOK], f16); nc.sync.dma_start(cosm[:], COSM[:])
        sinm = const.tile([64, TOK], f16); nc.sync.dma_start(sinm[:], SINM[:])
        cosx = const.tile([64, TOK], f16); nc.sync.dma_start(cosx[:], COSX[:])
        sinx = const.tile([64, TOK], f16); nc.sync.dma_start(sinx[:], SINX[:])
        maskc = const.tile([MEM, B, MEM], f16)
        nc.sync.dma_start(maskc[:], MASKC[:])
        scratch32 = const.tile([128, 128], f32)
        nc.vector.memset(scratch32[:], 1.0)
        ones16 = const.tile([128, 1], f16)
        nc.vector.tensor_copy(ones16[:], scratch32[:, 0:1])
        onesrow = const.tile([1, 128], f32r)
        nc.vector.tensor_copy(onesrow[:], scratch32[0:1, :])
        ident32 = const.tile([128, 128], f32)
        make_identity(nc, ident32)
        epst = const.tile([128, 1], f32)
        nc.vector.memset(epst[:], EPS)

        # ---- resident weights
        def wload(Wd, n_slabs, tag):
            w = wres.tile([128, n_slabs, KT, 128], f16, tag=tag, name=tag)
            for n in range(n_slabs):
                nc.sync.dma_start(w[:, n], Wd[n])
            return w

        wmr = wload(WM, 2, "wmr"); wor = wload(WO, 2, "wor")
        wqr = wload(WQ, 2, "wqr"); wkr = wload(WK, 2, "wkr")
        wkmr = wload(WKM, 2, "wkmr")
        wvr = wres.tile([128, KT, 256], f16, tag="wvr", name="wvr")
        nc.sync.dma_start(wvr[:], WV[:].rearrange("k p f -> p k f"))
        wvmr = wres.tile([128, KT, 256], f16, tag="wvmr", name="wvmr")
        nc.sync.dma_start(wvmr[:], WVM[:].rearrange("k p f -> p k f"))

        a_sh = aoutp.tile([128, 2, TOK], f16, tag="aout", name="a_init")
        nc.sync.dma_start(a_sh[:], OM0T[:].rearrange("k p t -> p k t"))

        def mm(p, lhsT, rhs, start, stop):
            nc.tensor.matmul(p, lhsT, rhs, start=start, stop=stop)

        def rowproj_stage(Wr, a_tile, writers):
            """Row-parallel 256->2048 proj of a_tile [128,2,TOK] fp16; psum
            quartets staged fp16; writers[qi] gives the DRAM dst AP for
            quartet qi."""
            for qi, wfn in enumerate(writers):
                stq = stage.tile([128, 4, TOK], f16, tag="stage", name="stq")
                for j in range(4):
                    n = 4 * qi + j
                    p = psA.tile([128, TOK], f32, tag="pa", name="pn")
                    mm(p[:], Wr[:, 0, n, :], a_tile[:, 0, :], True, False)
                    mm(p[:], Wr[:, 1, n, :], a_tile[:, 1, :], False, True)
                    if j % 2 == 0:
                        nc.scalar.copy(stq[:, j, :], p[:])
                    else:
                        nc.vector.tensor_copy(stq[:, j, :], p[:])
                nc.sync.dma_start(wfn(), stq[:])

        def allreduce(in_d, out_d):
            if COLL:
                nc.gpsimd.collective_compute(
                    "AllReduce", AluOp.add, replica_groups=rg,
                    ins=[in_d[:].opt()], outs=[out_d[:].opt()])
            else:
                nc.sync.dma_start(out_d[:], in_d[:])

        def reduce_scatter(in_d, out_ap, tag):
            rsout = dram.tile([FPC, TOK], f16, name=f"rsout_{tag}")
            if COLL:
                nc.gpsimd.collective_compute(
                    "ReduceScatter", AluOp.add, replica_groups=rg,
                    ins=[in_d[:].opt()], outs=[rsout[:].opt()])
            else:
                nc.sync.dma_start(rsout[:], in_d[0:FPC, :])
            nc.scalar.dma_start(out_ap, rsout[:])

        def bc_from_psum(ssq_ps, kind, name):
            """[1,TOK] psum -> broadcast [128,TOK] -> rstd (Sqrt+recip) or
            plain reciprocal; returns [128,TOK] f32 SBUF tile."""
            r = smol.tile([1, TOK], f32r, tag="smr", name=f"r_{name}", bufs=2)
            nc.vector.tensor_copy(r[:], ssq_ps[:])
            pbc = psF.tile([128, TOK], f32, tag="pf", name=f"pbc_{name}")
            mm(pbc[:], onesrow[:], r[:], True, True)
            bc = scr.tile([128, TOK], f32, tag=f"bc_{kind}", name=f"bc_{name}",
                          bufs=1 if kind == "rstd" else 2)
            if kind == "rstd":
                sq = scr.tile([128, TOK], f32, tag="sqt", name=f"sq_{name}",
                              bufs=1)
                nc.scalar.activation(sq[:], pbc[:], AFT.Sqrt,
                                     bias=epst[:], scale=1.0 / DIM)
                nc.vector.reciprocal_approx_fast(bc[:], sq[:])
            else:
                nc.vector.reciprocal_approx_fast(bc[:], pbc[:])
            return bc

        def rope2(dst_fn, src01, cosT, sinT):
            """src01: two psum tiles [128, TOK]; dst_fn(h, ri) -> fp16 AP."""
            for h in range(2):
                ph = src01[h]
                r, i = ph[0:64, :], ph[64:128, :]
                t1 = scr.tile([64, TOK], f32, tag="t1", name="t1", bufs=2)
                t2 = scr.tile([64, TOK], f32, tag="t1", name="t2", bufs=2)
                nc.vector.tensor_mul(t1[:], r, cosT[:])
                nc.vector.tensor_mul(t2[:], i, sinT[:])
                nc.vector.tensor_sub(dst_fn(h, 0), t1[:], t2[:])
                t3 = scr.tile([64, TOK], f32, tag="t1", name="t3", bufs=2)
                t4 = scr.tile([64, TOK], f32, tag="t1", name="t4", bufs=2)
                nc.vector.tensor_mul(t3[:], r, sinT[:])
                nc.vector.tensor_mul(t4[:], i, cosT[:])
                nc.vector.tensor_add(dst_fn(h, 1), t3[:], t4[:])

        def colproj2_psums(Wr, rhs_tile, pool, tag, k0, k1, ps=None,
                           stop=True):
            """2048 -> 256 col proj over k in [k0,k1); returns 2 psum tiles."""
            outs = []
            for n in range(2):
                if ps is None:
                    p = pool.tile([128, TOK], f32, tag=tag, name=f"pp{n}")
                else:
                    p = ps[n]
                for k in range(k0, k1):
                    mm(p[:], Wr[:, n, k, :], rhs_tile[:, k, :], k == k0 and k0 == 0,
                       stop and k == k1 - 1)
                outs.append(p)
            return outs

        def vproj_tok(lhs_tile, Wr, v, bs, k0, k1, ps=None, stop=True):
            """v[tok, b, 256] += x[k-tile, b-block]^T @ W[k-tile] over k."""
            outs = []
            for bi, b in enumerate(bs):
                if ps is None:
                    p = psA.tile([128, 256], f32, tag="pa", name=f"pv{b}")
                else:
                    p = ps[bi]
                for k in range(k0, k1):
                    mm(p[:], lhs_tile[:, k, b * 128:(b + 1) * 128],
                       Wr[:, k, :], k == k0 and k0 == 0, stop and k == k1 - 1)
                outs.append(p)
                if stop:
                    nc.scalar.copy(v[:, b, :], p[:])
            return outs

        # DRAM comm buffers
        def mk_arbufs(tag):
            ins = [dram.tile([128, 8, TOK], f16, name=f"ari_{tag}_{i}")
                   for i in range(2)]
            outs = [dram.tile([128, 8, TOK], f16, addr_space="Shared",
                              name=f"aro_{tag}_{i}") for i in range(2)]
            return ins, outs

        for t in range(NCH):
            # x chunk (fp16) + streamed FFN weights; issued early on sync
            x16 = xpool.tile([128, KT, TOK], f16, tag="x", name="x16")
            nc.sync.dma_start(x16[:],
                              XT[t].rearrange("(k p) t2 -> p k t2", p=128))
            w1t = [w13r.tile([128, KT, 128], f16, tag="w13", name=f"w1t{n}",
                             bufs=6) for n in range(HKT)]
            w3t = [w13r.tile([128, KT, 128], f16, tag="w13b", name=f"w3t{n}",
                             bufs=6) for n in range(HKT)]
            for n in range(HKT):
                nc.sync.dma_start(w1t[n][:], W1[n])
                nc.sync.dma_start(w3t[n][:], W3[n])

            # ---- 1. om1 partials = a_sh @ wm rows; single 2MB AR1
            arin1 = dram.tile([128, KT, TOK], f16, name=f"ari1_{t}")
            arout1 = dram.tile([128, KT, TOK], f16, addr_space="Shared",
                               name=f"aro1_{t}")
            rowproj_stage(wmr, a_sh,
                          [lambda q=q: arin1[:, q * 4:(q + 1) * 4, :]
                           for q in range(4)])
            allreduce(arin1[:], arout1[:])

            # ---- 2. yo partials for prev chunk (fills AR1 window); RS goes
            # on the CC queue between AR1 and AR2. yin layout: p-major per
            # 256-feature shard so staging is contiguous and RS scatters
            # correct shards.
            if t > 0:
                yin = dram.tile([NC, 2, 128, TOK], f16, name=f"yin_{t}")
                rowproj_stage(wor, a_sh,
                              [lambda q=q: yin[2 * q:2 * q + 2]
                               .rearrange("s u p t2 -> p s u t2")
                               for q in range(4)])
                reduce_scatter(yin, YO[t - 1], f"y{t - 1}")

            # ---- 3. x-side q/k projections + rope (AR1 window)
            qT = qkpool.tile([128, 2, TOK], f16, tag="qT", name="qT")
            qps = colproj2_psums(wqr, x16, psA, "pa", 0, KT)
            rope2(lambda h, ri: qT[ri * 64:(ri + 1) * 64, h, :],
                  qps, cosx, sinx)
            kall = qkpool.tile([128, 2, B, 2 * MEM], f16, tag="kall",
                               name="kall")
            kps = colproj2_psums(wkr, x16, psA, "pa", 0, KT)
            rope2(lambda h, ri: kall[ri * 64:(ri + 1) * 64, h, :, MEM:],
                  kps, cosx, sinx)

            # ---- 4. om1 lands in k-quarters
            om1 = ompool.tile([128, KT, TOK], f16, tag="om", name="om1")
            for qt in range(4):
                nc.sync.dma_start(om1[:, qt * 4:(qt + 1) * 4, :],
                                  arout1[:, qt * 4:(qt + 1) * 4, :])

            # ---- 5. FFN up; ssq1 via gpsimd squares + interleaved ones-mms
            ssq1 = psN.tile([1, TOK], f32, tag="pn", name="ssq1")
            sqs = []
            for k in range(KT):
                sq = scr.tile([128, TOK], f16, tag="sq16", name="sq", bufs=3)
                nc.gpsimd.tensor_mul(sq[:], om1[:, k, :], om1[:, k, :])
                sqs.append(sq)
            g = gpool.tile([128, HKT, TOK], f16, tag="g", name="g")
            bc1_l = [None]
            for n in range(HKT):
                p1 = psA.tile([128, TOK], f32, tag="pa", name="p1")
                for k in range(KT):
                    mm(p1[:], w1t[n][:, k, :], om1[:, k, :], k == 0,
                       k == KT - 1)
                p3 = psA.tile([128, TOK], f32, tag="pa", name="p3")
                for k in range(KT):
                    mm(p3[:], w3t[n][:, k, :], om1[:, k, :], k == 0,
                       k == KT - 1)
                if n == 0:
                    for k in range(KT):
                        mm(ssq1[:], ones16[:], sqs[k][:], k == 0, k == KT - 1)
                    bc1_l[0] = bc_from_psum(ssq1, "rstd", f"bc1_{t}")
                s1 = scr.tile([128, TOK], f16, tag="s1t", name="s1", bufs=2)
                nc.vector.tensor_mul(s1[:], p1[:], bc1_l[0][:])
                sil = scr.tile([128, TOK], f16, tag="silt", name="sil", bufs=1)
                nc.scalar.activation(sil[:], s1[:], AFT.Silu)
                m_ = scr.tile([128, TOK], f16, tag="s1t", name="m_", bufs=2)
                nc.vector.tensor_mul(m_[:], p3[:], sil[:])
                nc.gpsimd.tensor_mul(g[:, n, :], m_[:], bc1_l[0][:])

            # ---- 6. FFN down + residual; single 2MB AR2
            arin2 = dram.tile([128, KT, TOK], f16, name=f"ari2_{t}")
            arout2 = dram.tile([128, KT, TOK], f16, addr_space="Shared",
                               name=f"aro2_{t}")
            for q in range(4):
                w2p = []
                for j in range(4):
                    w2t = w2r.tile([128, HKT, 128], f16, tag="w2r",
                                   name="w2t", bufs=3)
                    nc.sync.dma_start(w2t[:], W2[4 * q + j])
                    w2p.append(w2t)
                stq = stage.tile([128, 4, TOK], f16, tag="stage", name="st2")
                for j in range(4):
                    nf = 4 * q + j
                    p = psA.tile([128, TOK], f32, tag="pa", name="pd")
                    for k in range(HKT):
                        mm(p[:], w2p[j][:, k, :], g[:, k, :],
                           k == 0, k == HKT - 1)
                    nc.vector.scalar_tensor_tensor(
                        stq[:, j, :], om1[:, nf, :], 1.0 / NC, p[:],
                        op0=AluOp.mult, op1=AluOp.add)
                nc.sync.dma_start(arin2[:, q * 4:(q + 1) * 4, :], stq[:])
            allreduce(arin2[:], arout2[:])

            # ---- 7. x-side v (AR2 window)
            vx = vpool.tile([128, B, 256], f16, tag="vx", name="vx")
            vproj_tok(x16, wvr, vx, [0, 1, 2, 3], 0, KT)

            # ---- 8. om2 lands in k-quarters; mem-side K/V + rstd2
            om2 = ompool.tile([128, KT, TOK], f16, tag="om", name="om2")
            for qt in range(4):
                nc.sync.dma_start(om2[:, qt * 4:(qt + 1) * 4, :],
                                  arout2[:, qt * 4:(qt + 1) * 4, :])

            ssq2 = psN.tile([1, TOK], f32, tag="pn", name="ssq2")
            sq2s = []
            for k in range(KT):
                sq = scr.tile([128, TOK], f16, tag="sq16", name="sq2", bufs=3)
                nc.gpsimd.tensor_mul(sq[:], om2[:, k, :], om2[:, k, :])
                sq2s.append(sq)
            kmps = [psF.tile([128, TOK], f32, tag="pf", name=f"km{n}")
                    for n in range(2)]
            for n in range(2):
                for k in range(KT):
                    mm(kmps[n][:], wkmr[:, n, k, :], om2[:, k, :], k == 0,
                       k == KT - 1)
                for k in range(8 * n, 8 * n + 8):
                    mm(ssq2[:], ones16[:], sq2s[k][:], k == 0, k == KT - 1)
            bc2 = bc_from_psum(ssq2, "rstd", f"bc2_{t}")
            rope2(lambda h, ri: kall[ri * 64:(ri + 1) * 64, h, :, 0:MEM],
                  kmps, cosm, sinm)
            vm = vpool.tile([128, B, 256], f16, tag="vm", name="vm")
            vproj_tok(om2, wvmr, vm, [0, 1, 2, 3], 0, KT)

            # rstd2 per mem-token: transpose bc2 blocks -> [128, B]
            rstd2T = smol.tile([128, B], f32, tag="r2T", name="r2T")
            for b in range(B):
                ptr = psA.tile([128, 128], f32, tag="pa", name="ptr")
                nc.tensor.transpose(ptr[:], bc2[:, b * 128:(b + 1) * 128],
                                    ident32[:])
                nc.vector.tensor_copy(rstd2T[:, b:b + 1], ptr[:, 0:1])

            # ---- 9. attention; denominators in one [1,TOK] psum; 1/den via
            # broadcast + approx reciprocal; scaled mem-e in separate tile.
            aout = aoutp.tile([128, 2, TOK], f16, tag="aout", name=f"aout_{t}")
            for h in range(HPC):
                eT = scr.tile([128, B, 2, MEM], f16, tag="eT", name="eT",
                              bufs=1)
                eTm = scr.tile([128, B, MEM], f16, tag="eTm", name="eTm",
                               bufs=2)
                for b in range(B):
                    ps = psA.tile([128, 2, MEM], f32, tag="pa", name="ps")
                    mm(ps[:, 0, :], kall[:, h, b, 0:MEM],
                       qT[:, h, b * 128:(b + 1) * 128], True, True)
                    mm(ps[:, 1, :], kall[:, h, b, MEM:],
                       qT[:, h, b * 128:(b + 1) * 128], True, True)
                    sT = scr.tile([128, 2, MEM], f32, tag="sT", name="sT",
                                  bufs=2)
                    nc.vector.tensor_scalar_mul(sT[:, 0, :], ps[:, 0, :],
                                                rstd2T[:, b:b + 1])
                    nc.vector.tensor_add(sT[:, 1, :], ps[:, 1, :],
                                         maskc[:, b, :])
                    nc.scalar.activation(eT[:, b, :, :], sT[:, :, :],
                                         AFT.Exp)
                    nc.gpsimd.tensor_scalar_mul(eTm[:, b, :], eT[:, b, 0, :],
                                                rstd2T[:, b:b + 1])
                pden = psN.tile([1, TOK], f32, tag="pn", name="pden")
                for b in range(B):
                    mm(pden[:, b * 128:(b + 1) * 128], ones16[:],
                       eT[:, b, 0, :], True, False)
                    mm(pden[:, b * 128:(b + 1) * 128], ones16[:],
                       eT[:, b, 1, :], False, True)
                rb = bc_from_psum(pden, "recip", f"rb{h}_{t}")
                for b in range(B):
                    po = psA.tile([128, 128], f32, tag="pa", name="po")
                    mm(po[:], vm[:, b, h * 128:(h + 1) * 128],
                       eTm[:, b, :], True, False)
                    mm(po[:], vx[:, b, h * 128:(h + 1) * 128],
                       eT[:, b, 1, :], False, True)
                    nc.vector.tensor_mul(aout[:, h, b * 128:(b + 1) * 128],
                                         po[:], rb[:, b * 128:(b + 1) * 128])
            a_sh = aout

        # final yo for last chunk
        yin = dram.tile([DIM, TOK], f16, name="yin_last")
        rowproj_stage(wor, a_sh,
                      [lambda q=q: yin[q * 512:(q + 1) * 512, :]
                       .rearrange("(j p) t2 -> p j t2", p=128)
                       for q in range(4)])
        reduce_scatter(yin, YO[NCH - 1], "ylast")
        es.close()

    nc.compile()
    return nc


def _get_runtime():
    if "nc" not in _RUNTIME:
        _RUNTIME["nc"] = _build()
    return _RUNTIME["nc"]


def _assemble(results):
    out = np.zeros((B, SEQ, DIM), np.float32)
    for c in range(NC):
        yo = np.asarray(results[c]["YO"], np.float32)  # [NCH, FPC, TOK]
        y = yo.reshape(NCH, FPC, B, MEM).transpose(2, 0, 3, 1)
        out[:, :, c * FPC:(c + 1) * FPC] = y.reshape(B, SEQ, FPC)
    return out


def kernel(**inputs):
    from concourse.bass_utils import run_bass_kernel_spmd
    nc = _get_runtime()
    in_maps = _prepare(inputs)
    res = run_bass_kernel_spmd(nc, in_maps, core_ids=list(range(NC)),
                               trace=False)
    return _assemble(res.results)


if __name__ == "__main__":
    _build()
    print("build ok")


# revision 16
# speedup vs baseline: 1.1287x; 1.0173x over previous
"""nn_MemoryAttention TP8 Trainium2 kernel, v3.

8 NeuronCores, T-layout activations [feature, token] (512 token cols =
4 batch x 128). Attention output stays head-sharded (2 heads = 256 features
per core); wm and wo are ROW-parallel on that shard; per-chunk recurrence
needs two AllReduces (om1 = a_sh @ wm, om2 = om1 + FFN) plus a per-chunk
ReduceScatter for yo = a_sh @ wo.

v3 changes vs v2:
- All weights and activations fp16 (incl. wm/wo/aout); f32 only in psum,
  softmax/norm scalar chains.
- x/mem projection weights SBUF-resident (no per-chunk reloads).
- FFN up-projection k-tiles 0-7 computed during the AR1 wire time of
  k-tiles 8-15 ("spill" partial psums to SBUF fp16, resume + add after the
  second AR half lands). Same trick hides AR2 behind KM/VM prefills.
- v projections computed directly in token-partition layout (weight as the
  moving operand), killing the per-chunk PE transposes.
- softmax/rms epilogues: psum-accumulated denominators, broadcast first,
  then reciprocal_approx_fast on the [128, 512] tile (the serial [1,512]
  DVE reciprocal was 3.3us each).
- yo ReduceScatter queued between AR1 and AR2 so it never delays the
  om-recurrence collectives.
"""
import numpy as np

DIM = 2048; NH = 16; HD = 128; MEM = 128; SEQ = 2048; B = 4; HID = 5632
EPS = 1e-5
NC = 8
HPC = NH // NC          # 2 heads per core
FPC = DIM // NC         # 256 features per core
HIDP = 768              # padded per-core FFN hidden (704 -> 768)
NCH = SEQ // MEM        # 16 chunks
TOK = B * MEM           # 512 token columns
KT = DIM // 128         # 16 feature k-tiles
HKT = HIDP // 128       # 6 hidden k-tiles
MASKV = -60.0

_RUNTIME = {}
COLL = True   # False: replace collectives with local DMAs (timing diagnostic)


def _head_perm():
    p = np.concatenate([np.arange(0, HD, 2), np.arange(1, HD, 2)])
    return np.concatenate([h * HD + p for h in range(NH)])


def _slab(w):
    """Column-shard weight [2048, C] -> [C//128, 128, KT, 128] (stationary)."""
    C = w.shape[1]
    return np.ascontiguousarray(
        w.reshape(KT, 128, C // 128, 128).transpose(2, 1, 0, 3))


def _slab_row(w):
    """Row-shard weight [256, 2048] -> [2, 128, 16, 128] (k-major)."""
    return np.ascontiguousarray(w.reshape(2, 128, 16, 128))


def _slab_mov(w):
    """Moving-operand weight [2048, 256] -> [KT, 128, 256]."""
    return np.ascontiguousarray(w.reshape(KT, 128, 256))


def _slab_w2(w2):
    """[768, 2048] -> [16, 128, HKT, 128]."""
    return np.ascontiguousarray(
        w2.reshape(HKT, 128, 16, 128).transpose(2, 1, 0, 3))


def _trunc22(x):
    u = np.ascontiguousarray(x, np.float32).view(np.uint32)
    return ((u + np.uint32(0x200)) & np.uint32(0xFFFFFC00)).view(np.float32)


def _prepare(inputs):
    perm = _head_perm()
    scale = 1.0 / np.sqrt(HD)
    fwv = np.asarray(inputs["ffn_norm_w"], np.float32)
    mwv = np.asarray(inputs["mem_norm_w"], np.float32)
    wq = np.asarray(inputs["wq"])[:, perm] * scale
    wk = np.asarray(inputs["wk"])[:, perm]
    wkm = (mwv[:, None] * np.asarray(inputs["wkm"]))[:, perm]
    wvm = mwv[:, None] * np.asarray(inputs["wvm"])
    wv = np.asarray(inputs["wv"])
    wm = np.asarray(inputs["wm"]); wo = np.asarray(inputs["wo"])
    w1 = np.zeros((DIM, NC * HIDP), np.float32)
    w3 = np.zeros((DIM, NC * HIDP), np.float32)
    w2 = np.zeros((NC * HIDP, DIM), np.float32)
    w1s = fwv[:, None] * np.asarray(inputs["w1"])
    w3s = fwv[:, None] * np.asarray(inputs["w3"])
    for c in range(NC):
        w1[:, c * HIDP:c * HIDP + 704] = w1s[:, c * 704:(c + 1) * 704]
        w3[:, c * HIDP:c * HIDP + 704] = w3s[:, c * 704:(c + 1) * 704]
        w2[c * HIDP:c * HIDP + 704] = np.asarray(inputs["w2"])[c * 704:(c + 1) * 704]
    fc = np.asarray(inputs["freqs_cos"]); fs = np.asarray(inputs["freqs_sin"])
    cos_mem = _trunc22(np.tile(fc[0:MEM].T, (1, B)))      # [64, 512]
    sin_mem = _trunc22(np.tile(fs[0:MEM].T, (1, B)))
    cos_x = _trunc22(np.tile(fc[MEM:2 * MEM].T, (1, B)))
    sin_x = _trunc22(np.tile(fs[MEM:2 * MEM].T, (1, B)))
    # causal mask for chunk-key columns only, transposed: [128 k, 4 b, 128 q]
    mask1 = np.zeros((MEM, MEM), np.float32)
    for i in range(MEM):
        mask1[i, i + 1:] = MASKV
    maskc = np.ascontiguousarray(
        np.broadcast_to(mask1.T[:, None, :], (MEM, B, MEM))).astype(np.float16)
    x = np.asarray(inputs["x"])
    xT = np.ascontiguousarray(
        x.reshape(B, NCH, MEM, DIM).transpose(1, 3, 0, 2)
        .reshape(NCH, DIM, TOK)).astype(np.float16)
    om0 = np.asarray(inputs["origin_mem"])
    omT0 = om0.transpose(2, 0, 1).reshape(DIM, TOK)
    in_maps = []
    for c in range(NC):
        hsl = slice(c * FPC, (c + 1) * FPC)
        hidsl = slice(c * HIDP, (c + 1) * HIDP)
        in_maps.append({
            "WM": _slab_row(wm[hsl, :]).astype(np.float16),
            "WO": _slab_row(wo[hsl, :]).astype(np.float16),
            "WKM": _slab(wkm[:, hsl]).astype(np.float16),
            "WVM": _slab_mov(wvm[:, hsl]).astype(np.float16),
            "WQ": _slab(wq[:, hsl]).astype(np.float16),
            "WK": _slab(wk[:, hsl]).astype(np.float16),
            "WV": _slab_mov(wv[:, hsl]).astype(np.float16),
            "W1": _slab(w1[:, hidsl]).astype(np.float16),
            "W3": _slab(w3[:, hidsl]).astype(np.float16),
            "W2": _slab_w2(w2[hidsl, :]).astype(np.float16),
            "XT": xT,
            "OM0T": omT0[c * FPC:(c + 1) * FPC].reshape(2, 128, TOK)
                    .astype(np.float16),
            "COSM": cos_mem.astype(np.float16), "SINM": sin_mem.astype(np.float16),
            "COSX": cos_x.astype(np.float16), "SINX": sin_x.astype(np.float16),
            "MASKC": maskc,
        })
    return in_maps


def _build():
    import concourse.bacc as bacc
    import concourse.tile as tile
    import concourse.mybir as mybir
    from concourse.masks import make_identity
    from contextlib import ExitStack

    dt = mybir.dt
    AluOp = mybir.AluOpType
    AFT = mybir.ActivationFunctionType
    f32, f32r, f16 = dt.float32, dt.float32r, dt.float16

    nc = bacc.Bacc("TRN2", target_bir_lowering=False, debug=False,
                   num_devices=NC)

    def din(name, shape, dtype=f16):
        return nc.dram_tensor(name, shape, dtype, kind="ExternalInput")

    WM = din("WM", [2, 128, KT, 128]); WO = din("WO", [2, 128, KT, 128])
    WKM = din("WKM", [2, 128, KT, 128])
    WVM = din("WVM", [KT, 128, 256])
    WQ = din("WQ", [2, 128, KT, 128]); WK = din("WK", [2, 128, KT, 128])
    WV = din("WV", [KT, 128, 256])
    W1 = din("W1", [HKT, 128, KT, 128])
    W3 = din("W3", [HKT, 128, KT, 128])
    W2 = din("W2", [KT, 128, HKT, 128])
    XT = din("XT", [NCH, DIM, TOK])
    OM0T = din("OM0T", [2, 128, TOK])
    COSM = din("COSM", [64, TOK]); SINM = din("SINM", [64, TOK])
    COSX = din("COSX", [64, TOK]); SINX = din("SINX", [64, TOK])
    MASKC = din("MASKC", [MEM, B, MEM])
    YO = nc.dram_tensor("YO", [NCH, FPC, TOK], f16, kind="ExternalOutput")

    rg = [list(range(NC))]

    with tile.TileContext(nc) as tc:
        es = ExitStack()
        const = es.enter_context(tc.tile_pool(name="const", bufs=1))
        wres = es.enter_context(tc.tile_pool(name="wres", bufs=1))
        w13r = es.enter_context(tc.tile_pool(name="w13r", bufs=4))
        w2r = es.enter_context(tc.tile_pool(name="w2r", bufs=2))
        ompool = es.enter_context(tc.tile_pool(name="ompool", bufs=1))
        xpool = es.enter_context(tc.tile_pool(name="xpool", bufs=1))
        gpool = es.enter_context(tc.tile_pool(name="gpool", bufs=1))
        qkpool = es.enter_context(tc.tile_pool(name="qkpool", bufs=1))
        vpool = es.enter_context(tc.tile_pool(name="vpool", bufs=1))
        stage = es.enter_context(tc.tile_pool(name="stage", bufs=3))
        scr = es.enter_context(tc.tile_pool(name="scr", bufs=2))
        smol = es.enter_context(tc.tile_pool(name="smol", bufs=2))
        aoutp = es.enter_context(tc.tile_pool(name="aoutp", bufs=2))
        dram = es.enter_context(tc.tile_pool(name="dram", bufs=1, space="DRAM"))
        psA = es.enter_context(tc.tile_pool(name="psA", bufs=3, space="PSUM"))
        psF = es.enter_context(tc.tile_pool(name="psF", bufs=4, space="PSUM"))
        psN = es.enter_context(tc.tile_pool(name="psN", bufs=1, space="PSUM"))

        # ---- constants
        cosm = const.tile([64, TOK], f16); nc.sync.dma_start(cosm[:], COSM[:])
        sinm = const.tile([64, TOK], f16); nc.sync.dma_start(sinm[:], SINM[:])
        cosx = const.tile([64, TOK], f16); nc.sync.dma_start(cosx[:], COSX[:])
        sinx = const.tile([64, TOK], f16); nc.sync.dma_start(sinx[:], SINX[:])
        maskc = const.tile([MEM, B, MEM], f16)
        nc.sync.dma_start(maskc[:], MASKC[:])
        scratch32 = const.tile([128, 128], f32)
        nc.vector.memset(scratch32[:], 1.0)
        ones16 = const.tile([128, 1], f16)
        nc.vector.tensor_copy(ones16[:], scratch32[:, 0:1])
        onesrow = const.tile([1, 128], f32r)
        nc.vector.tensor_copy(onesrow[:], scratch32[0:1, :])
        ident32 = const.tile([128, 128], f32)
        make_identity(nc, ident32)
        epst = const.tile([128, 1], f32)
        nc.vector.memset(epst[:], EPS)

        # ---- resident weights
        def wload(Wd, n_slabs, tag):
            w = wres.tile([128, n_slabs, KT, 128], f16, tag=tag, name=tag)
            for n in range(n_slabs):
                nc.sync.dma_start(w[:, n], Wd[n])
            return w

        wmr = wload(WM, 2, "wmr"); wor = wload(WO, 2, "wor")
        wqr = wload(WQ, 2, "wqr"); wkr = wload(WK, 2, "wkr")
        wkmr = wload(WKM, 2, "wkmr")
        wvr = wres.tile([128, KT, 256], f16, tag="wvr", name="wvr")
        nc.sync.dma_start(wvr[:], WV[:].rearrange("k p f -> p k f"))
        wvmr = wres.tile([128, KT, 256], f16, tag="wvmr", name="wvmr")
        nc.sync.dma_start(wvmr[:], WVM[:].rearrange("k p f -> p k f"))

        a_sh = aoutp.tile([128, 2, TOK], f16, tag="aout", name="a_init")
        nc.sync.dma_start(a_sh[:], OM0T[:].rearrange("k p t -> p k t"))

        def mm(p, lhsT, rhs, start, stop):
            nc.tensor.matmul(p, lhsT, rhs, start=start, stop=stop)

        def rowproj_stage(Wr, a_tile, writers):
            """Row-parallel 256->2048 proj of a_tile [128,2,TOK] fp16; psum
            quartets staged fp16; writers[qi] gives the DRAM dst AP for
            quartet qi."""
            for qi, wfn in enumerate(writers):
                stq = stage.tile([128, 4, TOK], f16, tag="stage", name="stq")
                for j in range(4):
                    n = 4 * qi + j
                    p = psA.tile([128, TOK], f32, tag="pa", name="pn")
                    mm(p[:], Wr[:, 0, n, :], a_tile[:, 0, :], True, False)
                    mm(p[:], Wr[:, 1, n, :], a_tile[:, 1, :], False, True)
                    if j % 2 == 0:
                        nc.scalar.copy(stq[:, j, :], p[:])
                    else:
                        nc.vector.tensor_copy(stq[:, j, :], p[:])
                nc.sync.dma_start(wfn(), stq[:])

        def allreduce(in_d, out_d):
            if COLL:
                nc.gpsimd.collective_compute(
                    "AllReduce", AluOp.add, replica_groups=rg,
                    ins=[in_d[:].opt()], outs=[out_d[:].opt()])
            else:
                nc.sync.dma_start(out_d[:], in_d[:])

        def reduce_scatter(in_d, out_ap, tag):
            rsout = dram.tile([FPC, TOK], f16, name=f"rsout_{tag}")
            if COLL:
                nc.gpsimd.collective_compute(
                    "ReduceScatter", AluOp.add, replica_groups=rg,
                    ins=[in_d[:].opt()], outs=[rsout[:].opt()])
            else:
                nc.sync.dma_start(rsout[:], in_d[0:FPC, :])
            nc.scalar.dma_start(out_ap, rsout[:])

        def bc_from_psum(ssq_ps, kind, name):
            """[1,TOK] psum -> broadcast [128,TOK] -> rstd (Sqrt+recip) or
            plain reciprocal; returns [128,TOK] f32 SBUF tile."""
            r = smol.tile([1, TOK], f32r, tag="smr", name=f"r_{name}", bufs=2)
            nc.vector.tensor_copy(r[:], ssq_ps[:])
            pbc = psF.tile([128, TOK], f32, tag="pf", name=f"pbc_{name}")
            mm(pbc[:], onesrow[:], r[:], True, True)
            bc = scr.tile([128, TOK], f32, tag=f"bc_{kind}", name=f"bc_{name}",
                          bufs=1 if kind == "rstd" else 2)
            if kind == "rstd":
                sq = scr.tile([128, TOK], f32, tag="sqt", name=f"sq_{name}",
                              bufs=1)
                nc.scalar.activation(sq[:], pbc[:], AFT.Sqrt,
                                     bias=epst[:], scale=1.0 / DIM)
                nc.vector.reciprocal_approx_fast(bc[:], sq[:])
            else:
                nc.vector.reciprocal_approx_fast(bc[:], pbc[:])
            return bc

        def rope2(dst_fn, src01, cosT, sinT):
            """src01: two psum tiles [128, TOK]; dst_fn(h, ri) -> fp16 AP."""
            for h in range(2):
                ph = src01[h]
                r, i = ph[0:64, :], ph[64:128, :]
                t1 = scr.tile([64, TOK], f32, tag="t1", name="t1", bufs=2)
                t2 = scr.tile([64, TOK], f32, tag="t1", name="t2", bufs=2)
                nc.vector.tensor_mul(t1[:], r, cosT[:])
                nc.vector.tensor_mul(t2[:], i, sinT[:])
                nc.vector.tensor_sub(dst_fn(h, 0), t1[:], t2[:])
                t3 = scr.tile([64, TOK], f32, tag="t1", name="t3", bufs=2)
                t4 = scr.tile([64, TOK], f32, tag="t1", name="t4", bufs=2)
                nc.vector.tensor_mul(t3[:], r, sinT[:])
                nc.vector.tensor_mul(t4[:], i, cosT[:])
                nc.vector.tensor_add(dst_fn(h, 1), t3[:], t4[:])

        def colproj2_psums(Wr, rhs_tile, pool, tag, k0, k1, ps=None,
                           stop=True):
            """2048 -> 256 col proj over k in [k0,k1); returns 2 psum tiles."""
            outs = []
            for n in range(2):
                if ps is None:
                    p = pool.tile([128, TOK], f32, tag=tag, name=f"pp{n}")
                else:
                    p = ps[n]
                for k in range(k0, k1):
                    mm(p[:], Wr[:, n, k, :], rhs_tile[:, k, :], k == k0 and k0 == 0,
                       stop and k == k1 - 1)
                outs.append(p)
            return outs

        def vproj_tok(lhs_tile, Wr, v, bs, pool=None, tag="pa", scale=None):
            """v[tok, b, 256] = x[k-tile, b-block]^T @ W[k-tile]; optional
            per-partition (per-token) scale applied on the copy-out."""
            pool = pool or psA
            for b in bs:
                p = pool.tile([128, 256], f32, tag=tag, name=f"pv{b}")
                for k in range(KT):
                    mm(p[:], lhs_tile[:, k, b * 128:(b + 1) * 128],
                       Wr[:, k, :], k == 0, k == KT - 1)
                if scale is None:
                    nc.scalar.copy(v[:, b, :], p[:])
                else:
                    nc.scalar.activation(v[:, b, :], p[:], AFT.Copy,
                                         scale=scale[:, b:b + 1])

        # DRAM comm buffers
        def mk_arbufs(tag):
            ins = [dram.tile([128, 8, TOK], f16, name=f"ari_{tag}_{i}")
                   for i in range(2)]
            outs = [dram.tile([128, 8, TOK], f16, addr_space="Shared",
                              name=f"aro_{tag}_{i}") for i in range(2)]
            return ins, outs

        for t in range(NCH):
            # x chunk (fp16) + streamed FFN weights; issued early on sync
            x16 = xpool.tile([128, KT, TOK], f16, tag="x", name="x16")
            nc.scalar.dma_start(x16[:],
                                XT[t].rearrange("(k p) t2 -> p k t2", p=128))
            w1t = [w13r.tile([128, KT, 128], f16, tag="w13", name=f"w1t{n}",
                             bufs=6) for n in range(HKT)]
            w3t = [w13r.tile([128, KT, 128], f16, tag="w13b", name=f"w3t{n}",
                             bufs=6) for n in range(HKT)]
            for n in range(HKT):
                nc.gpsimd.dma_start(w1t[n][:], W1[n])
                nc.gpsimd.dma_start(w3t[n][:], W3[n])

            # ---- 1. om1 partials = a_sh @ wm rows; single 2MB AR1
            arin1 = dram.tile([128, KT, TOK], f16, name=f"ari1_{t}")
            arout1 = dram.tile([128, KT, TOK], f16, addr_space="Shared",
                               name=f"aro1_{t}")
            rowproj_stage(wmr, a_sh,
                          [lambda q=q: arin1[:, q * 4:(q + 1) * 4, :]
                           for q in range(4)])
            allreduce(arin1[:], arout1[:])

            # ---- 2. yo partials for prev chunk (fills AR1 window); RS goes
            # on the CC queue between AR1 and AR2. yin layout: p-major per
            # 256-feature shard so staging is contiguous and RS scatters
            # correct shards.
            if t > 0:
                yin = dram.tile([NC, 2, 128, TOK], f16, name=f"yin_{t}")
                rowproj_stage(wor, a_sh,
                              [lambda q=q: yin[2 * q:2 * q + 2]
                               .rearrange("s u p t2 -> p s u t2")
                               for q in range(4)])
                reduce_scatter(yin, YO[t - 1], f"y{t - 1}")

            # ---- 3. x-side q/k projections + rope (AR1 window)
            qT = qkpool.tile([128, 2, TOK], f16, tag="qT", name="qT")
            qps = colproj2_psums(wqr, x16, psA, "pa", 0, KT)
            rope2(lambda h, ri: qT[ri * 64:(ri + 1) * 64, h, :],
                  qps, cosx, sinx)
            kall = qkpool.tile([128, 2, B, 2 * MEM], f16, tag="kall",
                               name="kall")
            kps = colproj2_psums(wkr, x16, psA, "pa", 0, KT)
            rope2(lambda h, ri: kall[ri * 64:(ri + 1) * 64, h, :, MEM:],
                  kps, cosx, sinx)

            # ---- 4. om1 lands in k-quarters
            om1 = ompool.tile([128, KT, TOK], f16, tag="om", name="om1")
            for qt in range(4):
                nc.sync.dma_start(om1[:, qt * 4:(qt + 1) * 4, :],
                                  arout1[:, qt * 4:(qt + 1) * 4, :])

            # ---- 5. FFN up; ssq1 via gpsimd squares + interleaved ones-mms
            ssq1 = psN.tile([1, TOK], f32, tag="pn", name="ssq1")
            sqs = []
            for k in range(KT):
                sq = scr.tile([128, TOK], f16, tag="sq16", name="sq", bufs=4)
                if k % 2 == 0:
                    nc.scalar.activation(sq[:], om1[:, k, :], AFT.Square)
                else:
                    nc.vector.tensor_mul(sq[:], om1[:, k, :], om1[:, k, :])
                sqs.append(sq)
            g = gpool.tile([128, HKT, TOK], f16, tag="g", name="g")
            bc1_l = [None]
            pend = []

            def ffn_epilogue(n, p1, p3):
                s1 = scr.tile([128, TOK], f16, tag="s1t", name="s1", bufs=3)
                nc.vector.tensor_mul(s1[:], p1[:], bc1_l[0][:])
                sil = scr.tile([128, TOK], f16, tag="silt", name="sil", bufs=1)
                nc.scalar.activation(sil[:], s1[:], AFT.Silu)
                m_ = scr.tile([128, TOK], f16, tag="s1t", name="m_", bufs=3)
                nc.vector.tensor_mul(m_[:], p3[:], sil[:])
                nc.gpsimd.tensor_mul(g[:, n, :], m_[:], bc1_l[0][:])

            for n in range(HKT):
                p1 = psA.tile([128, TOK], f32, tag="pa", name="p1")
                for k in range(KT):
                    mm(p1[:], w1t[n][:, k, :], om1[:, k, :], k == 0,
                       k == KT - 1)
                p3 = psA.tile([128, TOK], f32, tag="pa", name="p3")
                for k in range(KT):
                    mm(p3[:], w3t[n][:, k, :], om1[:, k, :], k == 0,
                       k == KT - 1)
                if n < 2:
                    for k in range(8 * n, 8 * n + 8):
                        mm(ssq1[:], ones16[:], sqs[k][:], k == 0, k == KT - 1)
                    pend.append((n, p1, p3))
                    if n == 1:
                        bc1_l[0] = bc_from_psum(ssq1, "rstd", f"bc1_{t}")
                        for args in pend:
                            ffn_epilogue(*args)
                else:
                    ffn_epilogue(n, p1, p3)

            # ---- 6. FFN down + residual; single 2MB AR2
            arin2 = dram.tile([128, KT, TOK], f16, name=f"ari2_{t}")
            arout2 = dram.tile([128, KT, TOK], f16, addr_space="Shared",
                               name=f"aro2_{t}")
            for q in range(4):
                w2p = []
                for j in range(4):
                    w2t = w2r.tile([128, HKT, 128], f16, tag="w2r",
                                   name="w2t", bufs=3)
                    nc.gpsimd.dma_start(w2t[:], W2[4 * q + j])
                    w2p.append(w2t)
                stq = stage.tile([128, 4, TOK], f16, tag="stage", name="st2")
                for j in range(4):
                    nf = 4 * q + j
                    p = psA.tile([128, TOK], f32, tag="pa", name="pd")
                    for k in range(HKT):
                        mm(p[:], w2p[j][:, k, :], g[:, k, :],
                           k == 0, k == HKT - 1)
                    nc.vector.scalar_tensor_tensor(
                        stq[:, j, :], om1[:, nf, :], 1.0 / NC, p[:],
                        op0=AluOp.mult, op1=AluOp.add)
                nc.sync.dma_start(arin2[:, q * 4:(q + 1) * 4, :], stq[:])
            allreduce(arin2[:], arout2[:])

            # ---- 7. x-side v (AR2 window)
            vx = vpool.tile([128, B, 256], f16, tag="vx", name="vx")
            vproj_tok(x16, wvr, vx, [0, 1, 2, 3])

            # ---- 8. om2 lands in k-quarters; mem-side K/V + rstd2
            om2 = ompool.tile([128, KT, TOK], f16, tag="om", name="om2")
            for qt in range(4):
                nc.sync.dma_start(om2[:, qt * 4:(qt + 1) * 4, :],
                                  arout2[:, qt * 4:(qt + 1) * 4, :])

            ssq2 = psN.tile([1, TOK], f32, tag="pn", name="ssq2")
            sq2s = []
            for k in range(KT):
                sq = scr.tile([128, TOK], f16, tag="sq16", name="sq2", bufs=4)
                if k % 2 == 0:
                    nc.scalar.activation(sq[:], om2[:, k, :], AFT.Square)
                else:
                    nc.vector.tensor_mul(sq[:], om2[:, k, :], om2[:, k, :])
                sq2s.append(sq)
            kmps = [psF.tile([128, TOK], f32, tag="pf", name=f"km{n}")
                    for n in range(2)]
            for n in range(2):
                for k in range(KT):
                    mm(kmps[n][:], wkmr[:, n, k, :], om2[:, k, :], k == 0,
                       k == KT - 1)
                for k in range(8 * n, 8 * n + 8):
                    mm(ssq2[:], ones16[:], sq2s[k][:], k == 0, k == KT - 1)
            bc2 = bc_from_psum(ssq2, "rstd", f"bc2_{t}")
            rope2(lambda h, ri: kall[ri * 64:(ri + 1) * 64, h, :, 0:MEM],
                  kmps, cosm, sinm)

            # rstd2 per mem-token: transpose bc2 blocks -> [128, B]
            rstd2T = smol.tile([128, B], f32, tag="r2T", name="r2T")
            for b in range(B):
                ptr = psA.tile([128, 128], f32, tag="pa", name="ptr")
                nc.tensor.transpose(ptr[:], bc2[:, b * 128:(b + 1) * 128],
                                    ident32[:])
                nc.vector.tensor_copy(rstd2T[:, b:b + 1], ptr[:, 0:1])
            # memory v with the rstd2 fold applied at copy-out
            vm = vpool.tile([128, B, 256], f16, tag="vm", name="vm")
            vproj_tok(om2, wvmr, vm, [0, 1, 2, 3], pool=psF, tag="pf",
                      scale=rstd2T)

            # ---- 9. attention; denominators in one [1,TOK] psum; 1/den via
            # broadcast + approx reciprocal; scaled mem-e in separate tile.
            aout = aoutp.tile([128, 2, TOK], f16, tag="aout", name=f"aout_{t}")
            for h in range(HPC):
                eT = scr.tile([128, B, 2, MEM], f16, tag="eT", name="eT",
                              bufs=1)
                for b in range(B):
                    ps = psA.tile([128, 2, MEM], f32, tag="pa", name="ps")
                    mm(ps[:, 0, :], kall[:, h, b, 0:MEM],
                       qT[:, h, b * 128:(b + 1) * 128], True, True)
                    mm(ps[:, 1, :], kall[:, h, b, MEM:],
                       qT[:, h, b * 128:(b + 1) * 128], True, True)
                    sT = scr.tile([128, 2, MEM], f32, tag="sT", name="sT",
                                  bufs=2)
                    nc.vector.tensor_scalar_mul(sT[:, 0, :], ps[:, 0, :],
                                                rstd2T[:, b:b + 1])
                    nc.vector.tensor_add(sT[:, 1, :], ps[:, 1, :],
                                         maskc[:, b, :])
                    nc.scalar.activation(eT[:, b, :, :], sT[:, :, :],
                                         AFT.Exp)
                pden = psN.tile([1, TOK], f32, tag="pn", name="pden")
                for b in range(B):
                    mm(pden[:, b * 128:(b + 1) * 128], ones16[:],
                       eT[:, b, 0, :], True, False)
                    mm(pden[:, b * 128:(b + 1) * 128], ones16[:],
                       eT[:, b, 1, :], False, True)
                rb = bc_from_psum(pden, "recip", f"rb{h}_{t}")
                for b in range(B):
                    po = psA.tile([128, 128], f32, tag="pa", name="po")
                    mm(po[:], vm[:, b, h * 128:(h + 1) * 128],
                       eT[:, b, 0, :], True, False)
                    mm(po[:], vx[:, b, h * 128:(h + 1) * 128],
                       eT[:, b, 1, :], False, True)
                    nc.vector.tensor_mul(aout[:, h, b * 128:(b + 1) * 128],
                                         po[:], rb[:, b * 128:(b + 1) * 128])
            a_sh = aout

        # final yo for last chunk
        yin = dram.tile([DIM, TOK], f16, name="yin_last")
        rowproj_stage(wor, a_sh,
                      [lambda q=q: yin[q * 512:(q + 1) * 512, :]
                       .rearrange("(j p) t2 -> p j t2", p=128)
                       for q in range(4)])
        reduce_scatter(yin, YO[NCH - 1], "ylast")
        es.close()

    nc.compile()
    return nc


def _get_runtime():
    if "nc" not in _RUNTIME:
        _RUNTIME["nc"] = _build()
    return _RUNTIME["nc"]


def _assemble(results):
    out = np.zeros((B, SEQ, DIM), np.float32)
    for c in range(NC):
        yo = np.asarray(results[c]["YO"], np.float32)  # [NCH, FPC, TOK]
        y = yo.reshape(NCH, FPC, B, MEM).transpose(2, 0, 3, 1)
        out[:, :, c * FPC:(c + 1) * FPC] = y.reshape(B, SEQ, FPC)
    return out


def kernel(**inputs):
    from concourse.bass_utils import run_bass_kernel_spmd
    nc = _get_runtime()
    in_maps = _prepare(inputs)
    res = run_bass_kernel_spmd(nc, in_maps, core_ids=list(range(NC)),
                               trace=False)
    return _assemble(res.results)


if __name__ == "__main__":
    _build()
    print("build ok")


# revision 17
# speedup vs baseline: 1.1628x; 1.0302x over previous
"""nn_MemoryAttention TP8 Trainium2 kernel, v3.

8 NeuronCores, T-layout activations [feature, token] (512 token cols =
4 batch x 128). Attention output stays head-sharded (2 heads = 256 features
per core); wm and wo are ROW-parallel on that shard; per-chunk recurrence
needs two AllReduces (om1 = a_sh @ wm, om2 = om1 + FFN) plus a per-chunk
ReduceScatter for yo = a_sh @ wo.

v3 changes vs v2:
- All weights and activations fp16 (incl. wm/wo/aout); f32 only in psum,
  softmax/norm scalar chains.
- x/mem projection weights SBUF-resident (no per-chunk reloads).
- FFN up-projection k-tiles 0-7 computed during the AR1 wire time of
  k-tiles 8-15 ("spill" partial psums to SBUF fp16, resume + add after the
  second AR half lands). Same trick hides AR2 behind KM/VM prefills.
- v projections computed directly in token-partition layout (weight as the
  moving operand), killing the per-chunk PE transposes.
- softmax/rms epilogues: psum-accumulated denominators, broadcast first,
  then reciprocal_approx_fast on the [128, 512] tile (the serial [1,512]
  DVE reciprocal was 3.3us each).
- yo ReduceScatter queued between AR1 and AR2 so it never delays the
  om-recurrence collectives.
"""
import numpy as np

DIM = 2048; NH = 16; HD = 128; MEM = 128; SEQ = 2048; B = 4; HID = 5632
EPS = 1e-5
NC = 8
HPC = NH // NC          # 2 heads per core
FPC = DIM // NC         # 256 features per core
HIDP = 768              # padded per-core FFN hidden (704 -> 768)
NCH = SEQ // MEM        # 16 chunks
TOK = B * MEM           # 512 token columns
KT = DIM // 128         # 16 feature k-tiles
HKT = HIDP // 128       # 6 hidden k-tiles
MASKV = -60.0

_RUNTIME = {}
COLL = True   # False: replace collectives with local DMAs (timing diagnostic)


def _head_perm():
    p = np.concatenate([np.arange(0, HD, 2), np.arange(1, HD, 2)])
    return np.concatenate([h * HD + p for h in range(NH)])


def _slab(w):
    """Column-shard weight [2048, C] -> [C//128, 128, KT, 128] (stationary)."""
    C = w.shape[1]
    return np.ascontiguousarray(
        w.reshape(KT, 128, C // 128, 128).transpose(2, 1, 0, 3))


def _slab_row(w):
    """Row-shard weight [256, 2048] -> [2, 128, 16, 128] (k-major)."""
    return np.ascontiguousarray(w.reshape(2, 128, 16, 128))


def _slab_mov(w):
    """Moving-operand weight [2048, 256] -> [KT, 128, 256]."""
    return np.ascontiguousarray(w.reshape(KT, 128, 256))


def _slab_w2(w2):
    """[768, 2048] -> [16, 128, HKT, 128]."""
    return np.ascontiguousarray(
        w2.reshape(HKT, 128, 16, 128).transpose(2, 1, 0, 3))


def _trunc22(x):
    u = np.ascontiguousarray(x, np.float32).view(np.uint32)
    return ((u + np.uint32(0x200)) & np.uint32(0xFFFFFC00)).view(np.float32)


def _prepare(inputs):
    perm = _head_perm()
    scale = 1.0 / np.sqrt(HD)
    fwv = np.asarray(inputs["ffn_norm_w"], np.float32)
    mwv = np.asarray(inputs["mem_norm_w"], np.float32)
    wq = np.asarray(inputs["wq"])[:, perm] * scale
    wk = np.asarray(inputs["wk"])[:, perm]
    wkm = (mwv[:, None] * np.asarray(inputs["wkm"]))[:, perm]
    wvm = mwv[:, None] * np.asarray(inputs["wvm"])
    wv = np.asarray(inputs["wv"])
    wm = np.asarray(inputs["wm"]); wo = np.asarray(inputs["wo"])
    w1 = np.zeros((DIM, NC * HIDP), np.float32)
    w3 = np.zeros((DIM, NC * HIDP), np.float32)
    w2 = np.zeros((NC * HIDP, DIM), np.float32)
    w1s = fwv[:, None] * np.asarray(inputs["w1"])
    w3s = fwv[:, None] * np.asarray(inputs["w3"])
    for c in range(NC):
        w1[:, c * HIDP:c * HIDP + 704] = w1s[:, c * 704:(c + 1) * 704]
        w3[:, c * HIDP:c * HIDP + 704] = w3s[:, c * 704:(c + 1) * 704]
        w2[c * HIDP:c * HIDP + 704] = np.asarray(inputs["w2"])[c * 704:(c + 1) * 704]
    fc = np.asarray(inputs["freqs_cos"]); fs = np.asarray(inputs["freqs_sin"])
    cos_mem = _trunc22(np.tile(fc[0:MEM].T, (1, B)))      # [64, 512]
    sin_mem = _trunc22(np.tile(fs[0:MEM].T, (1, B)))
    cos_x = _trunc22(np.tile(fc[MEM:2 * MEM].T, (1, B)))
    sin_x = _trunc22(np.tile(fs[MEM:2 * MEM].T, (1, B)))
    # causal mask for chunk-key columns only, transposed: [128 k, 4 b, 128 q]
    mask1 = np.zeros((MEM, MEM), np.float32)
    for i in range(MEM):
        mask1[i, i + 1:] = MASKV
    maskc = np.ascontiguousarray(
        np.broadcast_to(mask1.T[:, None, :], (MEM, B, MEM))).astype(np.float16)
    x = np.asarray(inputs["x"])
    xT = np.ascontiguousarray(
        x.reshape(B, NCH, MEM, DIM).transpose(1, 3, 0, 2)
        .reshape(NCH, DIM, TOK)).astype(np.float16)
    om0 = np.asarray(inputs["origin_mem"])
    omT0 = om0.transpose(2, 0, 1).reshape(DIM, TOK)
    in_maps = []
    for c in range(NC):
        hsl = slice(c * FPC, (c + 1) * FPC)
        hidsl = slice(c * HIDP, (c + 1) * HIDP)
        in_maps.append({
            "WM": _slab_row(wm[hsl, :]).astype(np.float16),
            "WO": _slab_row(wo[hsl, :]).astype(np.float16),
            "WKM": _slab(wkm[:, hsl]).astype(np.float16),
            "WVM": _slab_mov(wvm[:, hsl]).astype(np.float16),
            "WQ": _slab(wq[:, hsl]).astype(np.float16),
            "WK": _slab(wk[:, hsl]).astype(np.float16),
            "WV": _slab_mov(wv[:, hsl]).astype(np.float16),
            "W1": _slab(w1[:, hidsl]).astype(np.float16),
            "W3": _slab(w3[:, hidsl]).astype(np.float16),
            "W2": _slab_w2(w2[hidsl, :]).astype(np.float16),
            "XT": xT,
            "OM0T": omT0[c * FPC:(c + 1) * FPC].reshape(2, 128, TOK)
                    .astype(np.float16),
            "COSM": cos_mem.astype(np.float16), "SINM": sin_mem.astype(np.float16),
            "COSX": cos_x.astype(np.float16), "SINX": sin_x.astype(np.float16),
            "MASKC": maskc,
        })
    return in_maps


def _build():
    import concourse.bacc as bacc
    import concourse.tile as tile
    import concourse.mybir as mybir
    from concourse.masks import make_identity
    from contextlib import ExitStack

    dt = mybir.dt
    AluOp = mybir.AluOpType
    AFT = mybir.ActivationFunctionType
    f32, f32r, f16 = dt.float32, dt.float32r, dt.float16

    nc = bacc.Bacc("TRN2", target_bir_lowering=False, debug=False,
                   num_devices=NC)

    def din(name, shape, dtype=f16):
        return nc.dram_tensor(name, shape, dtype, kind="ExternalInput")

    WM = din("WM", [2, 128, KT, 128]); WO = din("WO", [2, 128, KT, 128])
    WKM = din("WKM", [2, 128, KT, 128])
    WVM = din("WVM", [KT, 128, 256])
    WQ = din("WQ", [2, 128, KT, 128]); WK = din("WK", [2, 128, KT, 128])
    WV = din("WV", [KT, 128, 256])
    W1 = din("W1", [HKT, 128, KT, 128])
    W3 = din("W3", [HKT, 128, KT, 128])
    W2 = din("W2", [KT, 128, HKT, 128])
    XT = din("XT", [NCH, DIM, TOK])
    OM0T = din("OM0T", [2, 128, TOK])
    COSM = din("COSM", [64, TOK]); SINM = din("SINM", [64, TOK])
    COSX = din("COSX", [64, TOK]); SINX = din("SINX", [64, TOK])
    MASKC = din("MASKC", [MEM, B, MEM])
    YO = nc.dram_tensor("YO", [NCH, FPC, TOK], f16, kind="ExternalOutput")

    rg = [list(range(NC))]

    with tile.TileContext(nc) as tc:
        es = ExitStack()
        const = es.enter_context(tc.tile_pool(name="const", bufs=1))
        wres = es.enter_context(tc.tile_pool(name="wres", bufs=1))
        w13r = es.enter_context(tc.tile_pool(name="w13r", bufs=4))
        w2r = es.enter_context(tc.tile_pool(name="w2r", bufs=2))
        ompool = es.enter_context(tc.tile_pool(name="ompool", bufs=1))
        xpool = es.enter_context(tc.tile_pool(name="xpool", bufs=1))
        gpool = es.enter_context(tc.tile_pool(name="gpool", bufs=1))
        qkpool = es.enter_context(tc.tile_pool(name="qkpool", bufs=1))
        vpool = es.enter_context(tc.tile_pool(name="vpool", bufs=1))
        stage = es.enter_context(tc.tile_pool(name="stage", bufs=3))
        scr = es.enter_context(tc.tile_pool(name="scr", bufs=2))
        smol = es.enter_context(tc.tile_pool(name="smol", bufs=2))
        aoutp = es.enter_context(tc.tile_pool(name="aoutp", bufs=2))
        dram = es.enter_context(tc.tile_pool(name="dram", bufs=1, space="DRAM"))
        psA = es.enter_context(tc.tile_pool(name="psA", bufs=3, space="PSUM"))
        psF = es.enter_context(tc.tile_pool(name="psF", bufs=4, space="PSUM"))
        psN = es.enter_context(tc.tile_pool(name="psN", bufs=1, space="PSUM"))

        # ---- constants
        cosm = const.tile([64, TOK], f16); nc.sync.dma_start(cosm[:], COSM[:])
        sinm = const.tile([64, TOK], f16); nc.sync.dma_start(sinm[:], SINM[:])
        cosx = const.tile([64, TOK], f16); nc.sync.dma_start(cosx[:], COSX[:])
        sinx = const.tile([64, TOK], f16); nc.sync.dma_start(sinx[:], SINX[:])
        maskc = const.tile([MEM, B, MEM], f16)
        nc.sync.dma_start(maskc[:], MASKC[:])
        scratch32 = const.tile([128, 128], f32)
        nc.vector.memset(scratch32[:], 1.0)
        ones16 = const.tile([128, 1], f16)
        nc.vector.tensor_copy(ones16[:], scratch32[:, 0:1])
        onesrow = const.tile([1, 128], f32r)
        nc.vector.tensor_copy(onesrow[:], scratch32[0:1, :])
        ident32 = const.tile([128, 128], f32)
        make_identity(nc, ident32)
        epst = const.tile([128, 1], f32)
        nc.vector.memset(epst[:], EPS)

        # ---- resident weights
        def wload(Wd, n_slabs, tag):
            w = wres.tile([128, n_slabs, KT, 128], f16, tag=tag, name=tag)
            for n in range(n_slabs):
                nc.sync.dma_start(w[:, n], Wd[n])
            return w

        wmr = wload(WM, 2, "wmr"); wor = wload(WO, 2, "wor")
        wqr = wload(WQ, 2, "wqr"); wkr = wload(WK, 2, "wkr")
        wkmr = wload(WKM, 2, "wkmr")
        wvr = wres.tile([128, KT, 256], f16, tag="wvr", name="wvr")
        nc.sync.dma_start(wvr[:], WV[:].rearrange("k p f -> p k f"))
        wvmr = wres.tile([128, KT, 256], f16, tag="wvmr", name="wvmr")
        nc.sync.dma_start(wvmr[:], WVM[:].rearrange("k p f -> p k f"))

        a_sh = aoutp.tile([128, 2, TOK], f16, tag="aout", name="a_init")
        nc.sync.dma_start(a_sh[:], OM0T[:].rearrange("k p t -> p k t"))

        w1t = [wres.tile([128, KT, 128], f16, tag=f"w1r{n}", name=f"w1t{n}")
               for n in range(HKT)]
        w3t = [wres.tile([128, KT, 128], f16, tag=f"w3r{n}", name=f"w3t{n}")
               for n in range(HKT)]
        for n in range(HKT):
            nc.sync.dma_start(w1t[n][:], W1[n])
            nc.sync.dma_start(w3t[n][:], W3[n])

        def mm(p, lhsT, rhs, start, stop):
            nc.tensor.matmul(p, lhsT, rhs, start=start, stop=stop)

        def rowproj_stage(Wr, a_tile, writers):
            """Row-parallel 256->2048 proj of a_tile [128,2,TOK] fp16; psum
            quartets staged fp16; writers[qi] gives the DRAM dst AP for
            quartet qi."""
            for qi, wfn in enumerate(writers):
                stq = stage.tile([128, 4, TOK], f16, tag="stage", name="stq")
                for j in range(4):
                    n = 4 * qi + j
                    p = psA.tile([128, TOK], f32, tag="pa", name="pn")
                    mm(p[:], Wr[:, 0, n, :], a_tile[:, 0, :], True, False)
                    mm(p[:], Wr[:, 1, n, :], a_tile[:, 1, :], False, True)
                    if j % 2 == 0:
                        nc.scalar.copy(stq[:, j, :], p[:])
                    else:
                        nc.vector.tensor_copy(stq[:, j, :], p[:])
                nc.sync.dma_start(wfn(), stq[:])

        def allreduce(in_d, out_d):
            if COLL:
                nc.gpsimd.collective_compute(
                    "AllReduce", AluOp.add, replica_groups=rg,
                    ins=[in_d[:].opt()], outs=[out_d[:].opt()])
            else:
                nc.sync.dma_start(out_d[:], in_d[:])

        def reduce_scatter(in_d, out_ap, tag):
            rsout = dram.tile([FPC, TOK], f16, name=f"rsout_{tag}")
            if COLL:
                nc.gpsimd.collective_compute(
                    "ReduceScatter", AluOp.add, replica_groups=rg,
                    ins=[in_d[:].opt()], outs=[rsout[:].opt()])
            else:
                nc.sync.dma_start(rsout[:], in_d[0:FPC, :])
            nc.scalar.dma_start(out_ap, rsout[:])

        def bc_from_psum(ssq_ps, kind, name):
            """[1,TOK] psum -> broadcast [128,TOK] -> rstd (Sqrt+recip) or
            plain reciprocal; returns [128,TOK] f32 SBUF tile."""
            r = smol.tile([1, TOK], f32r, tag="smr", name=f"r_{name}", bufs=2)
            nc.vector.tensor_copy(r[:], ssq_ps[:])
            pbc = psF.tile([128, TOK], f32, tag="pf", name=f"pbc_{name}")
            mm(pbc[:], onesrow[:], r[:], True, True)
            bc = scr.tile([128, TOK], f32, tag=f"bc_{kind}", name=f"bc_{name}",
                          bufs=1 if kind == "rstd" else 2)
            if kind == "rstd":
                sq = scr.tile([128, TOK], f32, tag="sqt", name=f"sq_{name}",
                              bufs=1)
                nc.scalar.activation(sq[:], pbc[:], AFT.Sqrt,
                                     bias=epst[:], scale=1.0 / DIM)
                nc.vector.reciprocal_approx_fast(bc[:], sq[:])
            else:
                nc.vector.reciprocal_approx_fast(bc[:], pbc[:])
            return bc

        def rope2(dst_fn, src01, cosT, sinT):
            """src01: two psum tiles [128, TOK]; dst_fn(h, ri) -> fp16 AP."""
            for h in range(2):
                ph = src01[h]
                r, i = ph[0:64, :], ph[64:128, :]
                t1 = scr.tile([64, TOK], f32, tag="t1", name="t1", bufs=2)
                t2 = scr.tile([64, TOK], f32, tag="t1", name="t2", bufs=2)
                nc.vector.tensor_mul(t1[:], r, cosT[:])
                nc.vector.tensor_mul(t2[:], i, sinT[:])
                nc.vector.tensor_sub(dst_fn(h, 0), t1[:], t2[:])
                t3 = scr.tile([64, TOK], f32, tag="t1", name="t3", bufs=2)
                t4 = scr.tile([64, TOK], f32, tag="t1", name="t4", bufs=2)
                nc.vector.tensor_mul(t3[:], r, sinT[:])
                nc.vector.tensor_mul(t4[:], i, cosT[:])
                nc.vector.tensor_add(dst_fn(h, 1), t3[:], t4[:])

        def colproj2_psums(Wr, rhs_tile, pool, tag, k0, k1, ps=None,
                           stop=True):
            """2048 -> 256 col proj over k in [k0,k1); returns 2 psum tiles."""
            outs = []
            for n in range(2):
                if ps is None:
                    p = pool.tile([128, TOK], f32, tag=tag, name=f"pp{n}")
                else:
                    p = ps[n]
                for k in range(k0, k1):
                    mm(p[:], Wr[:, n, k, :], rhs_tile[:, k, :], k == k0 and k0 == 0,
                       stop and k == k1 - 1)
                outs.append(p)
            return outs

        def vproj_tok(lhs_tile, Wr, v, bs, pool=None, tag="pa", scale=None):
            """v[tok, b, 256] = x[k-tile, b-block]^T @ W[k-tile]; optional
            per-partition (per-token) scale applied on the copy-out."""
            pool = pool or psA
            for b in bs:
                p = pool.tile([128, 256], f32, tag=tag, name=f"pv{b}")
                for k in range(KT):
                    mm(p[:], lhs_tile[:, k, b * 128:(b + 1) * 128],
                       Wr[:, k, :], k == 0, k == KT - 1)
                if scale is None:
                    nc.scalar.copy(v[:, b, :], p[:])
                else:
                    nc.scalar.activation(v[:, b, :], p[:], AFT.Copy,
                                         scale=scale[:, b:b + 1])

        # DRAM comm buffers
        def mk_arbufs(tag):
            ins = [dram.tile([128, 8, TOK], f16, name=f"ari_{tag}_{i}")
                   for i in range(2)]
            outs = [dram.tile([128, 8, TOK], f16, addr_space="Shared",
                              name=f"aro_{tag}_{i}") for i in range(2)]
            return ins, outs

        for t in range(NCH):
            # x chunk (fp16) + streamed FFN weights; issued early on sync
            x16 = xpool.tile([128, KT, TOK], f16, tag="x", name="x16")
            nc.scalar.dma_start(x16[:],
                                XT[t].rearrange("(k p) t2 -> p k t2", p=128))

            # ---- 1. om1 partials = a_sh @ wm rows; single 2MB AR1
            arin1 = dram.tile([128, KT, TOK], f16, name=f"ari1_{t}")
            arout1 = dram.tile([128, KT, TOK], f16, addr_space="Shared",
                               name=f"aro1_{t}")
            rowproj_stage(wmr, a_sh,
                          [lambda q=q: arin1[:, q * 4:(q + 1) * 4, :]
                           for q in range(4)])
            allreduce(arin1[:], arout1[:])

            # ---- 2. yo partials for prev chunk (fills AR1 window); RS goes
            # on the CC queue between AR1 and AR2. yin layout: p-major per
            # 256-feature shard so staging is contiguous and RS scatters
            # correct shards.
            if t > 0:
                yin = dram.tile([NC, 2, 128, TOK], f16, name=f"yin_{t}")
                rowproj_stage(wor, a_sh,
                              [lambda q=q: yin[2 * q:2 * q + 2]
                               .rearrange("s u p t2 -> p s u t2")
                               for q in range(4)])
                reduce_scatter(yin, YO[t - 1], f"y{t - 1}")

            # ---- 3. x-side q/k projections + rope (AR1 window)
            qT = qkpool.tile([128, 2, TOK], f16, tag="qT", name="qT")
            qps = colproj2_psums(wqr, x16, psA, "pa", 0, KT)
            rope2(lambda h, ri: qT[ri * 64:(ri + 1) * 64, h, :],
                  qps, cosx, sinx)
            kall = qkpool.tile([128, 2, B, 2 * MEM], f16, tag="kall",
                               name="kall")
            kps = colproj2_psums(wkr, x16, psA, "pa", 0, KT)
            rope2(lambda h, ri: kall[ri * 64:(ri + 1) * 64, h, :, MEM:],
                  kps, cosx, sinx)

            # ---- 4. om1 lands in k-quarters
            om1 = ompool.tile([128, KT, TOK], f16, tag="om", name="om1")
            for qt in range(4):
                nc.sync.dma_start(om1[:, qt * 4:(qt + 1) * 4, :],
                                  arout1[:, qt * 4:(qt + 1) * 4, :])

            # ---- 5. FFN up; ssq1 via gpsimd squares + interleaved ones-mms
            ssq1 = psN.tile([1, TOK], f32, tag="pn", name="ssq1")
            sqs = []
            for k in range(KT):
                sq = scr.tile([128, TOK], f16, tag="sq16", name="sq", bufs=4)
                if k % 2 == 0:
                    nc.scalar.activation(sq[:], om1[:, k, :], AFT.Square)
                else:
                    nc.vector.tensor_mul(sq[:], om1[:, k, :], om1[:, k, :])
                sqs.append(sq)
            g = gpool.tile([128, HKT, TOK], f16, tag="g", name="g")
            bc1_l = [None]
            pend = []

            def ffn_epilogue(n, p1, p3):
                s1 = scr.tile([128, TOK], f16, tag="s1t", name="s1", bufs=3)
                nc.vector.tensor_mul(s1[:], p1[:], bc1_l[0][:])
                sil = scr.tile([128, TOK], f16, tag="silt", name="sil", bufs=1)
                nc.scalar.activation(sil[:], s1[:], AFT.Silu)
                m_ = scr.tile([128, TOK], f16, tag="s1t", name="m_", bufs=3)
                nc.vector.tensor_mul(m_[:], p3[:], sil[:])
                nc.gpsimd.tensor_mul(g[:, n, :], m_[:], bc1_l[0][:])

            for n in range(HKT):
                p1 = psA.tile([128, TOK], f32, tag="pa", name="p1")
                for k in range(KT):
                    mm(p1[:], w1t[n][:, k, :], om1[:, k, :], k == 0,
                       k == KT - 1)
                p3 = psA.tile([128, TOK], f32, tag="pa", name="p3")
                for k in range(KT):
                    mm(p3[:], w3t[n][:, k, :], om1[:, k, :], k == 0,
                       k == KT - 1)
                if n < 2:
                    for k in range(8 * n, 8 * n + 8):
                        mm(ssq1[:], ones16[:], sqs[k][:], k == 0, k == KT - 1)
                    pend.append((n, p1, p3))
                    if n == 1:
                        bc1_l[0] = bc_from_psum(ssq1, "rstd", f"bc1_{t}")
                        for args in pend:
                            ffn_epilogue(*args)
                else:
                    ffn_epilogue(n, p1, p3)

            # ---- 6. FFN down + residual; single 2MB AR2
            arin2 = dram.tile([128, KT, TOK], f16, name=f"ari2_{t}")
            arout2 = dram.tile([128, KT, TOK], f16, addr_space="Shared",
                               name=f"aro2_{t}")
            for q in range(4):
                w2p = []
                for j in range(4):
                    w2t = w2r.tile([128, HKT, 128], f16, tag="w2r",
                                   name="w2t", bufs=3)
                    nc.gpsimd.dma_start(w2t[:], W2[4 * q + j])
                    w2p.append(w2t)
                stq = stage.tile([128, 4, TOK], f16, tag="stage", name="st2")
                for j in range(4):
                    nf = 4 * q + j
                    p = psA.tile([128, TOK], f32, tag="pa", name="pd")
                    for k in range(HKT):
                        mm(p[:], w2p[j][:, k, :], g[:, k, :],
                           k == 0, k == HKT - 1)
                    nc.vector.scalar_tensor_tensor(
                        stq[:, j, :], om1[:, nf, :], 1.0 / NC, p[:],
                        op0=AluOp.mult, op1=AluOp.add)
                nc.sync.dma_start(arin2[:, q * 4:(q + 1) * 4, :], stq[:])
            allreduce(arin2[:], arout2[:])

            # ---- 7. x-side v (AR2 window)
            vx = vpool.tile([128, B, 256], f16, tag="vx", name="vx")
            vproj_tok(x16, wvr, vx, [0, 1, 2, 3])

            # ---- 8. om2 lands in k-quarters; mem-side K/V + rstd2
            om2 = ompool.tile([128, KT, TOK], f16, tag="om", name="om2")
            for qt in range(4):
                nc.sync.dma_start(om2[:, qt * 4:(qt + 1) * 4, :],
                                  arout2[:, qt * 4:(qt + 1) * 4, :])

            ssq2 = psN.tile([1, TOK], f32, tag="pn", name="ssq2")
            sq2s = []
            for k in range(KT):
                sq = scr.tile([128, TOK], f16, tag="sq16", name="sq2", bufs=4)
                if k % 2 == 0:
                    nc.scalar.activation(sq[:], om2[:, k, :], AFT.Square)
                else:
                    nc.vector.tensor_mul(sq[:], om2[:, k, :], om2[:, k, :])
                sq2s.append(sq)
            kmps = [psF.tile([128, TOK], f32, tag="pf", name=f"km{n}")
                    for n in range(2)]
            for n in range(2):
                for k in range(KT):
                    mm(kmps[n][:], wkmr[:, n, k, :], om2[:, k, :], k == 0,
                       k == KT - 1)
                for k in range(8 * n, 8 * n + 8):
                    mm(ssq2[:], ones16[:], sq2s[k][:], k == 0, k == KT - 1)
            bc2 = bc_from_psum(ssq2, "rstd", f"bc2_{t}")
            rope2(lambda h, ri: kall[ri * 64:(ri + 1) * 64, h, :, 0:MEM],
                  kmps, cosm, sinm)

            # rstd2 per mem-token: transpose bc2 blocks -> [128, B]
            rstd2T = smol.tile([128, B], f32, tag="r2T", name="r2T")
            for b in range(B):
                ptr = psA.tile([128, 128], f32, tag="pa", name="ptr")
                nc.tensor.transpose(ptr[:], bc2[:, b * 128:(b + 1) * 128],
                                    ident32[:])
                nc.vector.tensor_copy(rstd2T[:, b:b + 1], ptr[:, 0:1])
            # memory v with the rstd2 fold applied at copy-out
            vm = vpool.tile([128, B, 256], f16, tag="vm", name="vm")
            vproj_tok(om2, wvmr, vm, [0, 1, 2, 3], pool=psF, tag="pf",
                      scale=rstd2T)

            # ---- 9. attention; denominators in one [1,TOK] psum; 1/den via
            # broadcast + approx reciprocal; scaled mem-e in separate tile.
            aout = aoutp.tile([128, 2, TOK], f16, tag="aout", name=f"aout_{t}")
            for h in range(HPC):
                eT = scr.tile([128, B, 2, MEM], f16, tag="eT", name="eT",
                              bufs=1)
                for b in range(B):
                    ps = psA.tile([128, 2, MEM], f32, tag="pa", name="ps")
                    mm(ps[:, 0, :], kall[:, h, b, 0:MEM],
                       qT[:, h, b * 128:(b + 1) * 128], True, True)
                    mm(ps[:, 1, :], kall[:, h, b, MEM:],
                       qT[:, h, b * 128:(b + 1) * 128], True, True)
                    sT = scr.tile([128, 2, MEM], f32, tag="sT", name="sT",
                                  bufs=2)
                    nc.vector.tensor_scalar_mul(sT[:, 0, :], ps[:, 0, :],
                                                rstd2T[:, b:b + 1])
                    nc.vector.tensor_add(sT[:, 1, :], ps[:, 1, :],
                                         maskc[:, b, :])
                    nc.scalar.activation(eT[:, b, :, :], sT[:, :, :],
                                         AFT.Exp)
                pden = psN.tile([1, TOK], f32, tag="pn", name="pden")
                for b in range(B):
                    mm(pden[:, b * 128:(b + 1) * 128], ones16[:],
                       eT[:, b, 0, :], True, False)
                    mm(pden[:, b * 128:(b + 1) * 128], ones16[:],
                       eT[:, b, 1, :], False, True)
                rb = bc_from_psum(pden, "recip", f"rb{h}_{t}")
                for b in range(B):
                    po = psA.tile([128, 128], f32, tag="pa", name="po")
                    mm(po[:], vm[:, b, h * 128:(h + 1) * 128],
                       eT[:, b, 0, :], True, False)
                    mm(po[:], vx[:, b, h * 128:(h + 1) * 128],
                       eT[:, b, 1, :], False, True)
                    nc.vector.tensor_mul(aout[:, h, b * 128:(b + 1) * 128],
                                         po[:], rb[:, b * 128:(b + 1) * 128])
            a_sh = aout

        # final yo for last chunk
        yin = dram.tile([DIM, TOK], f16, name="yin_last")
        rowproj_stage(wor, a_sh,
                      [lambda q=q: yin[q * 512:(q + 1) * 512, :]
                       .rearrange("(j p) t2 -> p j t2", p=128)
                       for q in range(4)])
        reduce_scatter(yin, YO[NCH - 1], "ylast")
        es.close()

    nc.compile()
    return nc


def _get_runtime():
    if "nc" not in _RUNTIME:
        _RUNTIME["nc"] = _build()
    return _RUNTIME["nc"]


def _assemble(results):
    out = np.zeros((B, SEQ, DIM), np.float32)
    for c in range(NC):
        yo = np.asarray(results[c]["YO"], np.float32)  # [NCH, FPC, TOK]
        y = yo.reshape(NCH, FPC, B, MEM).transpose(2, 0, 3, 1)
        out[:, :, c * FPC:(c + 1) * FPC] = y.reshape(B, SEQ, FPC)
    return out


def kernel(**inputs):
    from concourse.bass_utils import run_bass_kernel_spmd
    nc = _get_runtime()
    in_maps = _prepare(inputs)
    res = run_bass_kernel_spmd(nc, in_maps, core_ids=list(range(NC)),
                               trace=False)
    return _assemble(res.results)


if __name__ == "__main__":
    _build()
    print("build ok")


# revision 18
# speedup vs baseline: 1.2073x; 1.0382x over previous
"""nn_MemoryAttention TP8 Trainium2 kernel, v3.

8 NeuronCores, T-layout activations [feature, token] (512 token cols =
4 batch x 128). Attention output stays head-sharded (2 heads = 256 features
per core); wm and wo are ROW-parallel on that shard; per-chunk recurrence
needs two AllReduces (om1 = a_sh @ wm, om2 = om1 + FFN) plus a per-chunk
ReduceScatter for yo = a_sh @ wo.

v3 changes vs v2:
- All weights and activations fp16 (incl. wm/wo/aout); f32 only in psum,
  softmax/norm scalar chains.
- x/mem projection weights SBUF-resident (no per-chunk reloads).
- FFN up-projection k-tiles 0-7 computed during the AR1 wire time of
  k-tiles 8-15 ("spill" partial psums to SBUF fp16, resume + add after the
  second AR half lands). Same trick hides AR2 behind KM/VM prefills.
- v projections computed directly in token-partition layout (weight as the
  moving operand), killing the per-chunk PE transposes.
- softmax/rms epilogues: psum-accumulated denominators, broadcast first,
  then reciprocal_approx_fast on the [128, 512] tile (the serial [1,512]
  DVE reciprocal was 3.3us each).
- yo ReduceScatter queued between AR1 and AR2 so it never delays the
  om-recurrence collectives.
"""
import numpy as np

DIM = 2048; NH = 16; HD = 128; MEM = 128; SEQ = 2048; B = 4; HID = 5632
EPS = 1e-5
NC = 8
HPC = NH // NC          # 2 heads per core
FPC = DIM // NC         # 256 features per core
HIDP = 768              # padded per-core FFN hidden (704 -> 768)
NCH = SEQ // MEM        # 16 chunks
TOK = B * MEM           # 512 token columns
KT = DIM // 128         # 16 feature k-tiles
HKT = HIDP // 128       # 6 hidden k-tiles
MASKV = -60.0

_RUNTIME = {}
COLL = True   # False: replace collectives with local DMAs (timing diagnostic)


def _head_perm():
    p = np.concatenate([np.arange(0, HD, 2), np.arange(1, HD, 2)])
    return np.concatenate([h * HD + p for h in range(NH)])


def _slab(w):
    """Column-shard weight [2048, C] -> [C//128, 128, KT, 128] (stationary)."""
    C = w.shape[1]
    return np.ascontiguousarray(
        w.reshape(KT, 128, C // 128, 128).transpose(2, 1, 0, 3))


def _slab_row(w):
    """Row-shard weight [256, 2048] -> [2, 128, 16, 128] (k-major)."""
    return np.ascontiguousarray(w.reshape(2, 128, 16, 128))


def _slab_mov(w):
    """Moving-operand weight [2048, 256] -> [KT, 128, 256]."""
    return np.ascontiguousarray(w.reshape(KT, 128, 256))


def _slab_w2(w2):
    """[768, 2048] -> [16, 128, HKT, 128]."""
    return np.ascontiguousarray(
        w2.reshape(HKT, 128, 16, 128).transpose(2, 1, 0, 3))


def _trunc22(x):
    u = np.ascontiguousarray(x, np.float32).view(np.uint32)
    return ((u + np.uint32(0x200)) & np.uint32(0xFFFFFC00)).view(np.float32)


def _prepare(inputs):
    perm = _head_perm()
    scale = 1.0 / np.sqrt(HD)
    fwv = np.asarray(inputs["ffn_norm_w"], np.float32)
    mwv = np.asarray(inputs["mem_norm_w"], np.float32)
    wq = np.asarray(inputs["wq"])[:, perm] * scale
    wk = np.asarray(inputs["wk"])[:, perm]
    wkm = (mwv[:, None] * np.asarray(inputs["wkm"]))[:, perm]
    wvm = mwv[:, None] * np.asarray(inputs["wvm"])
    wv = np.asarray(inputs["wv"])
    wm = np.asarray(inputs["wm"]); wo = np.asarray(inputs["wo"])
    w1 = np.zeros((DIM, NC * HIDP), np.float32)
    w3 = np.zeros((DIM, NC * HIDP), np.float32)
    w2 = np.zeros((NC * HIDP, DIM), np.float32)
    w1s = fwv[:, None] * np.asarray(inputs["w1"])
    w3s = fwv[:, None] * np.asarray(inputs["w3"])
    for c in range(NC):
        w1[:, c * HIDP:c * HIDP + 704] = w1s[:, c * 704:(c + 1) * 704]
        w3[:, c * HIDP:c * HIDP + 704] = w3s[:, c * 704:(c + 1) * 704]
        w2[c * HIDP:c * HIDP + 704] = np.asarray(inputs["w2"])[c * 704:(c + 1) * 704]
    fc = np.asarray(inputs["freqs_cos"]); fs = np.asarray(inputs["freqs_sin"])
    cos_mem = _trunc22(np.tile(fc[0:MEM].T, (1, B)))      # [64, 512]
    sin_mem = _trunc22(np.tile(fs[0:MEM].T, (1, B)))
    cos_x = _trunc22(np.tile(fc[MEM:2 * MEM].T, (1, B)))
    sin_x = _trunc22(np.tile(fs[MEM:2 * MEM].T, (1, B)))
    # causal mask for chunk-key columns only, transposed: [128 k, 4 b, 128 q]
    mask1 = np.zeros((MEM, MEM), np.float32)
    for i in range(MEM):
        mask1[i, i + 1:] = MASKV
    maskc = np.ascontiguousarray(
        np.broadcast_to(mask1.T[:, None, :], (MEM, B, MEM))).astype(np.float16)
    x = np.asarray(inputs["x"])
    xT = np.ascontiguousarray(
        x.reshape(B, NCH, MEM, DIM).transpose(1, 3, 0, 2)
        .reshape(NCH, DIM, TOK)).astype(np.float16)
    om0 = np.asarray(inputs["origin_mem"])
    omT0 = om0.transpose(2, 0, 1).reshape(DIM, TOK)
    in_maps = []
    for c in range(NC):
        hsl = slice(c * FPC, (c + 1) * FPC)
        hidsl = slice(c * HIDP, (c + 1) * HIDP)
        in_maps.append({
            "WM": _slab_row(wm[hsl, :]).astype(np.float16),
            "WO": _slab_row(wo[hsl, :]).astype(np.float16),
            "WKM": _slab(wkm[:, hsl]).astype(np.float16),
            "WVM": _slab_mov(wvm[:, hsl]).astype(np.float16),
            "WQ": _slab(wq[:, hsl]).astype(np.float16),
            "WK": _slab(wk[:, hsl]).astype(np.float16),
            "WV": _slab_mov(wv[:, hsl]).astype(np.float16),
            "W1": _slab(w1[:, hidsl]).astype(np.float16),
            "W3": _slab(w3[:, hidsl]).astype(np.float16),
            "W2": _slab_w2(w2[hidsl, :]).astype(np.float16),
            "XT": xT,
            "OM0T": omT0[c * FPC:(c + 1) * FPC].reshape(2, 128, TOK)
                    .astype(np.float16),
            "COSM": cos_mem.astype(np.float16), "SINM": sin_mem.astype(np.float16),
            "COSX": cos_x.astype(np.float16), "SINX": sin_x.astype(np.float16),
            "MASKC": maskc,
        })
    return in_maps


def _build():
    import concourse.bacc as bacc
    import concourse.tile as tile
    import concourse.mybir as mybir
    from concourse.masks import make_identity
    from contextlib import ExitStack

    dt = mybir.dt
    AluOp = mybir.AluOpType
    AFT = mybir.ActivationFunctionType
    f32, f32r, f16 = dt.float32, dt.float32r, dt.float16

    nc = bacc.Bacc("TRN2", target_bir_lowering=False, debug=False,
                   num_devices=NC)

    def din(name, shape, dtype=f16):
        return nc.dram_tensor(name, shape, dtype, kind="ExternalInput")

    WM = din("WM", [2, 128, KT, 128]); WO = din("WO", [2, 128, KT, 128])
    WKM = din("WKM", [2, 128, KT, 128])
    WVM = din("WVM", [KT, 128, 256])
    WQ = din("WQ", [2, 128, KT, 128]); WK = din("WK", [2, 128, KT, 128])
    WV = din("WV", [KT, 128, 256])
    W1 = din("W1", [HKT, 128, KT, 128])
    W3 = din("W3", [HKT, 128, KT, 128])
    W2 = din("W2", [KT, 128, HKT, 128])
    XT = din("XT", [NCH, DIM, TOK])
    OM0T = din("OM0T", [2, 128, TOK])
    COSM = din("COSM", [64, TOK]); SINM = din("SINM", [64, TOK])
    COSX = din("COSX", [64, TOK]); SINX = din("SINX", [64, TOK])
    MASKC = din("MASKC", [MEM, B, MEM])
    YO = nc.dram_tensor("YO", [NCH, FPC, TOK], f16, kind="ExternalOutput")

    rg = [list(range(NC))]

    with tile.TileContext(nc) as tc:
        es = ExitStack()
        const = es.enter_context(tc.tile_pool(name="const", bufs=1))
        wres = es.enter_context(tc.tile_pool(name="wres", bufs=1))
        w13r = es.enter_context(tc.tile_pool(name="w13r", bufs=4))
        w2r = es.enter_context(tc.tile_pool(name="w2r", bufs=2))
        ompool = es.enter_context(tc.tile_pool(name="ompool", bufs=1))
        xpool = es.enter_context(tc.tile_pool(name="xpool", bufs=1))
        gpool = es.enter_context(tc.tile_pool(name="gpool", bufs=1))
        qkpool = es.enter_context(tc.tile_pool(name="qkpool", bufs=1))
        vpool = es.enter_context(tc.tile_pool(name="vpool", bufs=1))
        stage = es.enter_context(tc.tile_pool(name="stage", bufs=3))
        scr = es.enter_context(tc.tile_pool(name="scr", bufs=2))
        smol = es.enter_context(tc.tile_pool(name="smol", bufs=2))
        aoutp = es.enter_context(tc.tile_pool(name="aoutp", bufs=2))
        dram = es.enter_context(tc.tile_pool(name="dram", bufs=1, space="DRAM"))
        psA = es.enter_context(tc.tile_pool(name="psA", bufs=3, space="PSUM"))
        psF = es.enter_context(tc.tile_pool(name="psF", bufs=4, space="PSUM"))
        psN = es.enter_context(tc.tile_pool(name="psN", bufs=1, space="PSUM"))

        # ---- constants
        cosm = const.tile([64, TOK], f16); nc.sync.dma_start(cosm[:], COSM[:])
        sinm = const.tile([64, TOK], f16); nc.sync.dma_start(sinm[:], SINM[:])
        cosx = const.tile([64, TOK], f16); nc.sync.dma_start(cosx[:], COSX[:])
        sinx = const.tile([64, TOK], f16); nc.sync.dma_start(sinx[:], SINX[:])
        maskc = const.tile([MEM, B, MEM], f16)
        nc.sync.dma_start(maskc[:], MASKC[:])
        scratch32 = const.tile([128, 128], f32)
        nc.vector.memset(scratch32[:], 1.0)
        ones16 = const.tile([128, 1], f16)
        nc.vector.tensor_copy(ones16[:], scratch32[:, 0:1])
        onesrow = const.tile([1, 128], f32r)
        nc.vector.tensor_copy(onesrow[:], scratch32[0:1, :])
        ident32 = const.tile([128, 128], f32)
        make_identity(nc, ident32)
        epst = const.tile([128, 1], f32)
        nc.vector.memset(epst[:], EPS)

        # ---- resident weights
        def wload(Wd, n_slabs, tag):
            w = wres.tile([128, n_slabs, KT, 128], f16, tag=tag, name=tag)
            for n in range(n_slabs):
                nc.sync.dma_start(w[:, n], Wd[n])
            return w

        wmr = wload(WM, 2, "wmr"); wor = wload(WO, 2, "wor")
        wqr = wload(WQ, 2, "wqr"); wkr = wload(WK, 2, "wkr")
        wkmr = wload(WKM, 2, "wkmr")
        wvr = wres.tile([128, KT, 256], f16, tag="wvr", name="wvr")
        nc.sync.dma_start(wvr[:], WV[:].rearrange("k p f -> p k f"))
        wvmr = wres.tile([128, KT, 256], f16, tag="wvmr", name="wvmr")
        nc.sync.dma_start(wvmr[:], WVM[:].rearrange("k p f -> p k f"))

        a_sh = aoutp.tile([128, 2, TOK], f16, tag="aout", name="a_init")
        nc.sync.dma_start(a_sh[:], OM0T[:].rearrange("k p t -> p k t"))

        w1t = [wres.tile([128, KT, 128], f16, tag=f"w1r{n}", name=f"w1t{n}")
               for n in range(HKT)]
        w3t = [wres.tile([128, KT, 128], f16, tag=f"w3r{n}", name=f"w3t{n}")
               for n in range(HKT)]
        for n in range(HKT):
            nc.sync.dma_start(w1t[n][:], W1[n])
            nc.sync.dma_start(w3t[n][:], W3[n])

        def mm(p, lhsT, rhs, start, stop):
            nc.tensor.matmul(p, lhsT, rhs, start=start, stop=stop)

        def rowproj_stage(Wr, a_tile, writers):
            """Row-parallel 256->2048 proj of a_tile [128,2,TOK] fp16; psum
            quartets staged fp16; writers[qi] gives the DRAM dst AP for
            quartet qi."""
            for qi, wfn in enumerate(writers):
                stq = stage.tile([128, 4, TOK], f16, tag="stage", name="stq")
                for j in range(4):
                    n = 4 * qi + j
                    p = psA.tile([128, TOK], f32, tag="pa", name="pn")
                    mm(p[:], Wr[:, 0, n, :], a_tile[:, 0, :], True, False)
                    mm(p[:], Wr[:, 1, n, :], a_tile[:, 1, :], False, True)
                    if j % 2 == 0:
                        nc.scalar.copy(stq[:, j, :], p[:])
                    else:
                        nc.vector.tensor_copy(stq[:, j, :], p[:])
                nc.sync.dma_start(wfn(), stq[:])

        def allreduce(in_d, out_d):
            if COLL:
                nc.gpsimd.collective_compute(
                    "AllReduce", AluOp.add, replica_groups=rg,
                    ins=[in_d[:].opt()], outs=[out_d[:].opt()])
            else:
                nc.sync.dma_start(out_d[:], in_d[:])

        def reduce_scatter(in_d, out_ap, tag):
            rsout = dram.tile([FPC, TOK], f16, name=f"rsout_{tag}")
            if COLL:
                nc.gpsimd.collective_compute(
                    "ReduceScatter", AluOp.add, replica_groups=rg,
                    ins=[in_d[:].opt()], outs=[rsout[:].opt()])
            else:
                nc.sync.dma_start(rsout[:], in_d[0:FPC, :])
            nc.scalar.dma_start(out_ap, rsout[:])

        def bc_from_psum(ssq_ps, kind, name):
            """[1,TOK] psum -> broadcast [128,TOK] -> rstd (Sqrt+recip) or
            plain reciprocal; returns [128,TOK] f32 SBUF tile."""
            r = smol.tile([1, TOK], f32r, tag="smr", name=f"r_{name}", bufs=2)
            nc.vector.tensor_copy(r[:], ssq_ps[:])
            pbc = psF.tile([128, TOK], f32, tag="pf", name=f"pbc_{name}")
            mm(pbc[:], onesrow[:], r[:], True, True)
            bc = scr.tile([128, TOK], f32, tag=f"bc_{kind}", name=f"bc_{name}",
                          bufs=1 if kind == "rstd" else 2)
            if kind == "rstd":
                sq = scr.tile([128, TOK], f32, tag="sqt", name=f"sq_{name}",
                              bufs=1)
                nc.scalar.activation(sq[:], pbc[:], AFT.Sqrt,
                                     bias=epst[:], scale=1.0 / DIM)
                nc.vector.reciprocal_approx_fast(bc[:], sq[:])
            else:
                nc.vector.reciprocal_approx_fast(bc[:], pbc[:])
            return bc

        def rope2(dst_fn, src01, cosT, sinT):
            """src01: two psum tiles [128, TOK]; dst_fn(h, ri) -> fp16 AP."""
            for h in range(2):
                ph = src01[h]
                r, i = ph[0:64, :], ph[64:128, :]
                t1 = scr.tile([64, TOK], f32, tag="t1", name="t1", bufs=2)
                t2 = scr.tile([64, TOK], f32, tag="t1", name="t2", bufs=2)
                nc.vector.tensor_mul(t1[:], r, cosT[:])
                nc.vector.tensor_mul(t2[:], i, sinT[:])
                nc.vector.tensor_sub(dst_fn(h, 0), t1[:], t2[:])
                t3 = scr.tile([64, TOK], f32, tag="t1", name="t3", bufs=2)
                t4 = scr.tile([64, TOK], f32, tag="t1", name="t4", bufs=2)
                nc.vector.tensor_mul(t3[:], r, sinT[:])
                nc.vector.tensor_mul(t4[:], i, cosT[:])
                nc.vector.tensor_add(dst_fn(h, 1), t3[:], t4[:])

        def colproj2_psums(Wr, rhs_tile, pool, tag, k0, k1, ps=None,
                           stop=True):
            """2048 -> 256 col proj over k in [k0,k1); returns 2 psum tiles."""
            outs = []
            for n in range(2):
                if ps is None:
                    p = pool.tile([128, TOK], f32, tag=tag, name=f"pp{n}")
                else:
                    p = ps[n]
                for k in range(k0, k1):
                    mm(p[:], Wr[:, n, k, :], rhs_tile[:, k, :], k == k0 and k0 == 0,
                       stop and k == k1 - 1)
                outs.append(p)
            return outs

        def vproj_tok(lhs_tile, Wr, v, bs, pool=None, tag="pa", scale=None):
            """v[tok, b, 256] = x[k-tile, b-block]^T @ W[k-tile]; optional
            per-partition (per-token) scale applied on the copy-out."""
            pool = pool or psA
            for b in bs:
                p = pool.tile([128, 256], f32, tag=tag, name=f"pv{b}")
                for k in range(KT):
                    mm(p[:], lhs_tile[:, k, b * 128:(b + 1) * 128],
                       Wr[:, k, :], k == 0, k == KT - 1)
                if scale is None:
                    nc.scalar.copy(v[:, b, :], p[:])
                else:
                    nc.scalar.activation(v[:, b, :], p[:], AFT.Copy,
                                         scale=scale[:, b:b + 1])

        # DRAM comm buffers
        def mk_arbufs(tag):
            ins = [dram.tile([128, 8, TOK], f16, name=f"ari_{tag}_{i}")
                   for i in range(2)]
            outs = [dram.tile([128, 8, TOK], f16, addr_space="Shared",
                              name=f"aro_{tag}_{i}") for i in range(2)]
            return ins, outs

        for t in range(NCH):
            # x chunk (fp16) + streamed FFN weights; issued early on sync
            x16 = xpool.tile([128, KT, TOK], f16, tag="x", name="x16")
            nc.scalar.dma_start(x16[:],
                                XT[t].rearrange("(k p) t2 -> p k t2", p=128))

            # ---- 1. om1 partials = a_sh @ wm rows; single 2MB AR1
            arin1 = dram.tile([128, KT, TOK], f16, name=f"ari1_{t}")
            arout1 = dram.tile([128, KT, TOK], f16, addr_space="Shared",
                               name=f"aro1_{t}")
            rowproj_stage(wmr, a_sh,
                          [lambda q=q: arin1[:, q * 4:(q + 1) * 4, :]
                           for q in range(4)])
            allreduce(arin1[:], arout1[:])

            # ---- 2. yo partials for prev chunk (fills AR1 window); RS goes
            # on the CC queue between AR1 and AR2. yin layout: p-major per
            # 256-feature shard so staging is contiguous and RS scatters
            # correct shards.
            if t > 0:
                yin = dram.tile([NC, 2, 128, TOK], f16, name=f"yin_{t}")
                rowproj_stage(wor, a_sh,
                              [lambda q=q: yin[2 * q:2 * q + 2]
                               .rearrange("s u p t2 -> p s u t2")
                               for q in range(4)])
                reduce_scatter(yin, YO[t - 1], f"y{t - 1}")

            # ---- 3. x-side q/k projections + rope (AR1 window)
            qT = qkpool.tile([128, 2, TOK], f16, tag="qT", name="qT")
            qps = colproj2_psums(wqr, x16, psA, "pa", 0, KT)
            rope2(lambda h, ri: qT[ri * 64:(ri + 1) * 64, h, :],
                  qps, cosx, sinx)
            kall = qkpool.tile([128, 2, B, 2 * MEM], f16, tag="kall",
                               name="kall")
            kps = colproj2_psums(wkr, x16, psA, "pa", 0, KT)
            rope2(lambda h, ri: kall[ri * 64:(ri + 1) * 64, h, :, MEM:],
                  kps, cosx, sinx)

            # ---- 4. om1 lands in k-quarters
            om1 = ompool.tile([128, KT, TOK], f16, tag="om", name="om1")
            for qt in range(4):
                nc.sync.dma_start(om1[:, qt * 4:(qt + 1) * 4, :],
                                  arout1[:, qt * 4:(qt + 1) * 4, :])

            # ---- 5. FFN up; ssq1 via gpsimd squares + interleaved ones-mms
            ssq1 = psN.tile([1, TOK], f32, tag="pn", name="ssq1")
            sqs = []
            for k in range(KT):
                sq = scr.tile([128, TOK], f16, tag="sq16", name="sq", bufs=4)
                if k % 2 == 0:
                    nc.scalar.activation(sq[:], om1[:, k, :], AFT.Square)
                else:
                    nc.vector.tensor_mul(sq[:], om1[:, k, :], om1[:, k, :])
                sqs.append(sq)
            g = gpool.tile([128, HKT, TOK], f16, tag="g", name="g")
            for k in range(KT):
                mm(ssq1[:], ones16[:], sqs[k][:], k == 0, k == KT - 1)
            bc1 = bc_from_psum(ssq1, "rstd", f"bc1_{t}")
            for n in range(HKT):
                p1 = psA.tile([128, TOK], f32, tag="pa", name="p1")
                for k in range(KT):
                    mm(p1[:], w1t[n][:, k, :], om1[:, k, :], k == 0,
                       k == KT - 1)
                p3 = psA.tile([128, TOK], f32, tag="pa", name="p3")
                for k in range(KT):
                    mm(p3[:], w3t[n][:, k, :], om1[:, k, :], k == 0,
                       k == KT - 1)
                s1 = scr.tile([128, TOK], f16, tag="s1t", name="s1", bufs=3)
                nc.vector.tensor_mul(s1[:], p1[:], bc1[:])
                sil = scr.tile([128, TOK], f16, tag="silt", name="sil", bufs=1)
                nc.scalar.activation(sil[:], s1[:], AFT.Silu)
                m_ = scr.tile([128, TOK], f16, tag="s1t", name="m_", bufs=3)
                nc.vector.tensor_mul(m_[:], p3[:], sil[:])
                nc.gpsimd.tensor_mul(g[:, n, :], m_[:], bc1[:])

            # ---- 6. FFN down + residual; single 2MB AR2
            arin2 = dram.tile([128, KT, TOK], f16, name=f"ari2_{t}")
            arout2 = dram.tile([128, KT, TOK], f16, addr_space="Shared",
                               name=f"aro2_{t}")
            for q in range(4):
                w2p = []
                for j in range(4):
                    w2t = w2r.tile([128, HKT, 128], f16, tag="w2r",
                                   name="w2t", bufs=3)
                    nc.gpsimd.dma_start(w2t[:], W2[4 * q + j])
                    w2p.append(w2t)
                stq = stage.tile([128, 4, TOK], f16, tag="stage", name="st2")
                for j in range(4):
                    nf = 4 * q + j
                    p = psA.tile([128, TOK], f32, tag="pa", name="pd")
                    for k in range(HKT):
                        mm(p[:], w2p[j][:, k, :], g[:, k, :],
                           k == 0, k == HKT - 1)
                    nc.vector.scalar_tensor_tensor(
                        stq[:, j, :], om1[:, nf, :], 1.0 / NC, p[:],
                        op0=AluOp.mult, op1=AluOp.add)
                nc.sync.dma_start(arin2[:, q * 4:(q + 1) * 4, :], stq[:])
            allreduce(arin2[:], arout2[:])

            # ---- 7. x-side v (AR2 window)
            vx = vpool.tile([128, B, 256], f16, tag="vx", name="vx")
            vproj_tok(x16, wvr, vx, [0, 1, 2, 3])

            # ---- 8. om2 lands in k-quarters; mem-side K/V + rstd2
            om2 = ompool.tile([128, KT, TOK], f16, tag="om", name="om2")
            for qt in range(4):
                nc.sync.dma_start(om2[:, qt * 4:(qt + 1) * 4, :],
                                  arout2[:, qt * 4:(qt + 1) * 4, :])

            ssq2 = psN.tile([1, TOK], f32, tag="pn", name="ssq2")
            sq2s = []
            for k in range(KT):
                sq = scr.tile([128, TOK], f16, tag="sq16", name="sq2", bufs=4)
                if k % 2 == 0:
                    nc.scalar.activation(sq[:], om2[:, k, :], AFT.Square)
                else:
                    nc.vector.tensor_mul(sq[:], om2[:, k, :], om2[:, k, :])
                sq2s.append(sq)
            kmps = [psF.tile([128, TOK], f32, tag="pf", name=f"km{n}")
                    for n in range(2)]
            for n in range(2):
                for k in range(KT):
                    mm(kmps[n][:], wkmr[:, n, k, :], om2[:, k, :], k == 0,
                       k == KT - 1)
                for k in range(8 * n, 8 * n + 8):
                    mm(ssq2[:], ones16[:], sq2s[k][:], k == 0, k == KT - 1)
            bc2 = bc_from_psum(ssq2, "rstd", f"bc2_{t}")
            rope2(lambda h, ri: kall[ri * 64:(ri + 1) * 64, h, :, 0:MEM],
                  kmps, cosm, sinm)

            # rstd2 per mem-token: transpose bc2 blocks -> [128, B]
            rstd2T = smol.tile([128, B], f32, tag="r2T", name="r2T")
            for b in range(B):
                ptr = psA.tile([128, 128], f32, tag="pa", name="ptr")
                nc.tensor.transpose(ptr[:], bc2[:, b * 128:(b + 1) * 128],
                                    ident32[:])
                nc.vector.tensor_copy(rstd2T[:, b:b + 1], ptr[:, 0:1])
            # memory v with the rstd2 fold applied at copy-out
            vm = vpool.tile([128, B, 256], f16, tag="vm", name="vm")
            vproj_tok(om2, wvmr, vm, [0, 1, 2, 3], pool=psF, tag="pf",
                      scale=rstd2T)

            # ---- 9. attention; denominators in one [1,TOK] psum; 1/den via
            # broadcast + approx reciprocal; scaled mem-e in separate tile.
            aout = aoutp.tile([128, 2, TOK], f16, tag="aout", name=f"aout_{t}")
            for h in range(HPC):
                eT = scr.tile([128, B, 2, MEM], f16, tag="eT", name="eT",
                              bufs=1)
                for b in range(B):
                    ps = psA.tile([128, 2, MEM], f32, tag="pa", name="ps")
                    mm(ps[:, 0, :], kall[:, h, b, 0:MEM],
                       qT[:, h, b * 128:(b + 1) * 128], True, True)
                    mm(ps[:, 1, :], kall[:, h, b, MEM:],
                       qT[:, h, b * 128:(b + 1) * 128], True, True)
                    sT = scr.tile([128, 2, MEM], f32, tag="sT", name="sT",
                                  bufs=2)
                    nc.vector.tensor_scalar_mul(sT[:, 0, :], ps[:, 0, :],
                                                rstd2T[:, b:b + 1])
                    nc.vector.tensor_add(sT[:, 1, :], ps[:, 1, :],
                                         maskc[:, b, :])
                    nc.scalar.activation(eT[:, b, :, :], sT[:, :, :],
                                         AFT.Exp)
                pden = psN.tile([1, TOK], f32, tag="pn", name="pden")
                for b in range(B):
                    mm(pden[:, b * 128:(b + 1) * 128], ones16[:],
                       eT[:, b, 0, :], True, False)
                    mm(pden[:, b * 128:(b + 1) * 128], ones16[:],
                       eT[:, b, 1, :], False, True)
                rb = bc_from_psum(pden, "recip", f"rb{h}_{t}")
                for b in range(B):
                    po = psA.tile([128, 128], f32, tag="pa", name="po")
                    mm(po[:], vm[:, b, h * 128:(h + 1) * 128],
                       eT[:, b, 0, :], True, False)
                    mm(po[:], vx[:, b, h * 128:(h + 1) * 128],
                       eT[:, b, 1, :], False, True)
                    nc.vector.tensor_mul(aout[:, h, b * 128:(b + 1) * 128],
                                         po[:], rb[:, b * 128:(b + 1) * 128])
            a_sh = aout

        # final yo for last chunk
        yin = dram.tile([DIM, TOK], f16, name="yin_last")
        rowproj_stage(wor, a_sh,
                      [lambda q=q: yin[q * 512:(q + 1) * 512, :]
                       .rearrange("(j p) t2 -> p j t2", p=128)
                       for q in range(4)])
        reduce_scatter(yin, YO[NCH - 1], "ylast")
        es.close()

    nc.compile()
    return nc


def _get_runtime():
    if "nc" not in _RUNTIME:
        _RUNTIME["nc"] = _build()
    return _RUNTIME["nc"]


def _assemble(results):
    out = np.zeros((B, SEQ, DIM), np.float32)
    for c in range(NC):
        yo = np.asarray(results[c]["YO"], np.float32)  # [NCH, FPC, TOK]
        y = yo.reshape(NCH, FPC, B, MEM).transpose(2, 0, 3, 1)
        out[:, :, c * FPC:(c + 1) * FPC] = y.reshape(B, SEQ, FPC)
    return out


def kernel(**inputs):
    from concourse.bass_utils import run_bass_kernel_spmd
    nc = _get_runtime()
    in_maps = _prepare(inputs)
    res = run_bass_kernel_spmd(nc, in_maps, core_ids=list(range(NC)),
                               trace=False)
    return _assemble(res.results)


if __name__ == "__main__":
    _build()
    print("build ok")
